# revision 1
# baseline (speedup 1.0000x reference)
"""Trainium2 Bass kernel for a masked-attention block (MAB).

Computation (per batch element, all fp32):
    Q = X@Wq + bq ; K = Y@Wk + bk ; V = Y@Wv + bv
    logits = per-head Qh@Kh^T / 32, masked keys -> -inf, softmax over keys
    attn   = A @ Vh (concat heads)
    O1 = LN(Q + attn; g1,b1)
    O  = LN(O1 + relu(O1@Wo + bo); g2,b2)

Sharding: pure data-parallel, one batch element per NeuronCore (B=8 = 8 cores).

On-device dataflow is "feature-major": activations live in SBUF transposed
([model_dim -> 8x128 partitions, token -> free]).  With weights in natural
layout every matmul chains without any transposes:
    actT_out[n, t] = sum_d W[d, n] * actT_in[d, t]   (lhsT=W, rhs=actT_in)
Attention also chains: logitsT[k, q] from (lhsT=KT_h, rhs=QT_h) single
128-contraction; exp on ACT (mask folded in as a per-partition bias);
AV from (lhsT=V_natural, rhs=expT).  The softmax denominator and the
LayerNorm stats are partition-dim reductions done with all-ones stationary
matmuls (which also broadcast the result across partitions for free).
All matmuls use float32r (FP22 truncation) which runs at full PE rate for
moving free-dim >= 256.

The host transposes X/Y on the way in and the output on the way out, and
converts the bool mask into an additive f32 bias (0 / -1e4).
"""

import math
import numpy as np
from contextlib import ExitStack

import concourse.bass as bass
import concourse.mybir as mybir
import concourse.tile as tile
from concourse import bacc
from concourse.bass_utils import run_bass_kernel_spmd

P = 128
NX = 1024
NY = 1024
DIM = 1024
H = 8
KO = DIM // P          # 8 partition sub-tiles of the model dim
QC = 512               # moving-operand chunk (fp32 max free dim)
NQC = NX // QC         # 2
F32 = mybir.dt.float32
F32R = mybir.dt.float32r
BF16 = mybir.dt.bfloat16
# ldw-opt dedupes adjacent same-stationary LDWEIGHTS, but it is disabled in
# every production compile config here and we could not A/B-verify it on
# hardware before the time budget ran out — keep it off.
ENABLE_LDW_OPT = False
AF = mybir.ActivationFunctionType
ALU = mybir.AluOpType
SCALE = 1.0 / 32.0     # 1/sqrt(DIM)
EPS = 1e-5


def _r(ap):
    return ap.bitcast(F32R)


_LDW_PATCHED = False


def _patch_ldw_opt():
    """walrus ships with --enable-ldw-opt=false hardcoded; with our loop
    order same-stationary matmuls are adjacent, so deduping LDWEIGHTS is a
    large PE win.  Rewrite the flag on the walrus command line."""
    global _LDW_PATCHED
    if _LDW_PATCHED or not ENABLE_LDW_OPT:
        return
    import concourse.bass_utils as _bu
    _orig = _bu.run_command

    def _run_command(argv, **kwargs):
        argv = ["--enable-ldw-opt=true" if a == "--enable-ldw-opt=false" else a
                for a in argv]
        return _orig(argv, **kwargs)

    _bu.run_command = _run_command
    _LDW_PATCHED = True


def _build():
    _patch_ldw_opt()
    nc = bacc.Bacc("TRN2", target_bir_lowering=False, debug=False,
                   enable_asserts=False)

    # ---- DRAM I/O (per-core shapes) ----
    XT = nc.dram_tensor("XT", [DIM, NX], F32, kind="ExternalInput").ap()
    YT = nc.dram_tensor("YT", [DIM, NY], F32, kind="ExternalInput").ap()
    MB = nc.dram_tensor("MB", [NY], F32, kind="ExternalInput").ap()
    Wd = {}
    for w in ("Wq", "Wk", "Wv", "Wo"):
        Wd[w] = nc.dram_tensor(w, [DIM, DIM], F32, kind="ExternalInput").ap()
    Vecs = {}
    for vname in ("bq", "bk", "bv", "bo", "g1", "b1", "g2", "b2"):
        Vecs[vname] = nc.dram_tensor(vname, [DIM], F32, kind="ExternalInput").ap()
    OT = nc.dram_tensor("OT", [DIM, NX], F32, kind="ExternalOutput").ap()

    xt3 = XT.rearrange("(ko p) q -> p ko q", p=P)
    yt3 = YT.rearrange("(ko p) q -> p ko q", p=P)
    wq3 = Wd["Wq"].rearrange("(ko p) d -> p ko d", p=P)
    wk3 = Wd["Wk"].rearrange("(ko p) d -> p ko d", p=P)
    wv3 = Wd["Wv"].rearrange("(ko p) d -> p ko d", p=P)
    wo3 = Wd["Wo"].rearrange("(ko p) d -> p ko d", p=P)
    ot3 = OT.rearrange("(do p) q -> p do q", p=P)

    with tile.TileContext(nc) as tc:
        with ExitStack() as octx:
            const = octx.enter_context(tc.tile_pool(name="const", bufs=1))
            actp = octx.enter_context(tc.tile_pool(name="act", bufs=3))

            # ---- constants ----
            # walrus requires every writer of an fp32r-matmul operand to have
            # an fp32r-tagged output AP; memset can't write f32r, so round
            # the ones through a copy
            ones128 = const.tile([P, P], F32, tag="ones", name="ones128")
            ones_tmp = const.tile([P, P], F32, tag="onest", name="ones_tmp")
            nc.vector.memset(ones_tmp, 1.0)
            nc.vector.tensor_copy(_r(ones128), ones_tmp)
            ones_bf = const.tile([P, P], BF16, tag="onesbf", name="ones_bf")
            nc.vector.memset(ones_bf, 1.0)
            eps_sb = const.tile([P, 1], F32, tag="eps", name="eps_sb")
            nc.vector.memset(eps_sb, EPS)

            def vec_pko(name):
                t = const.tile([P, KO], F32, tag=f"v_{name}", name=f"{name}_sb")
                nc.sync.dma_start(t, Vecs[name].rearrange("(ko p) -> p ko", p=P))
                return t

            mb_sb = const.tile([P, KO], F32, tag="v_mb", name="mb_sb")
            nc.sync.dma_start(mb_sb, MB.rearrange("(ko p) -> p ko", p=P))
            bq_sb = vec_pko("bq")
            bk_sb = vec_pko("bk")
            bo_sb = vec_pko("bo")
            g1_sb = vec_pko("g1")
            b1_sb = vec_pko("b1")
            g2_sb = vec_pko("g2")
            b2_sb = vec_pko("b2")
            bv_sb = const.tile([1, DIM], F32, tag="v_bv", name="bv_sb")
            nc.sync.dma_start(_r(bv_sb),
                              _r(Vecs["bv"].rearrange("(one n) -> one n", one=1)))

            # ---- big feature-major activation tiles (rotating slots) ----
            qt = actp.tile([P, KO, NX], F32, tag="big", name="qt")
            ktm = actp.tile([P, KO, NY], F32, tag="big", name="ktm")
            vm = actp.tile([P, KO, DIM], BF16, tag="big", name="vm")

            # ================= Phase 1: Q, K, V projections =================
            with tc.tile_pool(name="io", bufs=1) as iop, \
                 tc.tile_pool(name="w1", bufs=2) as wp, \
                 tc.tile_pool(name="gp1", bufs=8, space="PSUM") as pp:
                xt = iop.tile([P, KO, NX], F32, tag="xt", name="xt")
                yt = iop.tile([P, KO, NY], F32, tag="yt", name="yt")
                for k in range(KO):
                    nc.sync.dma_start(_r(xt[:, k, :]), _r(xt3[:, k, :]))
                for k in range(KO):
                    nc.sync.dma_start(_r(yt[:, k, :]), _r(yt3[:, k, :]))

                def proj_featmajor(w3, rhs_sb, out_sb, bias_sb, label):
                    # out_sb[p, do, q] (+= bias[do*128+p]) = sum_k W[k, d] rhs[k, q]
                    # qc innermost: both uses of each stationary tile are
                    # back-to-back so ldw-opt can dedupe the LDWEIGHTS
                    for dg in range(2):
                        wt = wp.tile([P, KO, QC], F32, tag="w", name=f"w_{label}{dg}")
                        for k in range(KO):
                            nc.sync.dma_start(_r(wt[:, k, :]),
                                              _r(w3[:, k, dg * QC:(dg + 1) * QC]))
                        for d4 in range(4):
                            pss = [pp.tile([P, QC], F32, tag="ps",
                                           name=f"ps_{label}{dg}{d4}{qc}")
                                   for qc in range(NQC)]
                            for k in range(KO):
                                for qc in range(NQC):
                                    qs = slice(qc * QC, (qc + 1) * QC)
                                    nc.tensor.matmul(
                                        pss[qc],
                                        lhsT=_r(wt[:, k, d4 * P:(d4 + 1) * P]),
                                        rhs=_r(rhs_sb[:, k, qs]),
                                        start=(k == 0), stop=(k == KO - 1))
                            do = dg * 4 + d4
                            for qc in range(NQC):
                                qs = slice(qc * QC, (qc + 1) * QC)
                                nc.scalar.activation(
                                    _r(out_sb[:, do, qs]), pss[qc], AF.Identity,
                                    bias=bias_sb[:, do:do + 1], scale=1.0)

                proj_featmajor(wq3, xt, qt, bq_sb, "q")
                proj_featmajor(wk3, yt, ktm, bk_sb, "k")

                # V in natural (token-major) layout: V[y, n] = sum_k Y[y,k] Wv[k,n]
                # (bf16 output — only consumed by the AV matmul).  ng innermost
                # so each yt stationary tile is used twice back-to-back.
                wts = []
                for ng in range(2):
                    wt = wp.tile([P, KO, QC], F32, tag="w", name=f"w_v{ng}")
                    for k in range(KO):
                        nc.sync.dma_start(_r(wt[:, k, :]),
                                          _r(wv3[:, k, ng * QC:(ng + 1) * QC]))
                    wts.append(wt)
                for yo in range(KO):
                    pss = [pp.tile([P, QC], F32, tag="ps", name=f"ps_v{yo}{ng}")
                           for ng in range(2)]
                    for k in range(KO):
                        for ng in range(2):
                            nc.tensor.matmul(
                                pss[ng],
                                lhsT=_r(yt[:, k, yo * P:(yo + 1) * P]),
                                rhs=_r(wts[ng][:, k, :]),
                                start=(k == 0), stop=False)
                    for ng in range(2):
                        ns = slice(ng * QC, (ng + 1) * QC)
                        # fold per-free-dim bias bv with a K=1 ones matmul
                        nc.tensor.matmul(
                            pss[ng], lhsT=_r(ones128[0:1, :]), rhs=_r(bv_sb[:, ns]),
                            start=False, stop=True)
                        nc.scalar.copy(vm[:, yo, ns], pss[ng])

            # ================= Phase 2: attention =================
            with tc.tile_pool(name="zp", bufs=1) as zp:
                zt = zp.tile([P, KO, NX], F32, tag="z", name="zt")

                with tc.tile_pool(name="exp", bufs=20) as ep, \
                     tc.tile_pool(name="rcp", bufs=2) as rp, \
                     tc.tile_pool(name="lgp", bufs=2, space="PSUM") as lgp, \
                     tc.tile_pool(name="avp", bufs=1, space="PSUM") as avp, \
                     tc.tile_pool(name="rlp", bufs=1, space="PSUM") as rlp:

                    def logits_exp(h):
                        # logitsT[k, q] = sum_d KT_h[d, k] QT_h[d, q]; exp with
                        # mask bias per key (partition) and 1/32 scale.  The
                        # logits psum tile spans 2 banks so one ACT op covers
                        # the whole [128, 1024] key-slice.  exp output is bf16
                        # (feeds only the bf16 AV/denominator matmuls).
                        et = [ep.tile([P, NY], BF16, tag="exp", name=f"et{h}_{k}")
                              for k in range(KO)]
                        for kt in range(KO):
                            pl = lgp.tile([P, NX], F32, tag="lg",
                                          name=f"pl{h}{kt}")
                            for qc in range(NQC):
                                qs = slice(qc * QC, (qc + 1) * QC)
                                nc.tensor.matmul(
                                    pl[:, qs],
                                    lhsT=_r(ktm[:, h, kt * P:(kt + 1) * P]),
                                    rhs=_r(qt[:, h, qs]),
                                    start=True, stop=True)
                            nc.scalar.activation(
                                et[kt], pl, AF.Exp,
                                bias=mb_sb[:, kt:kt + 1], scale=SCALE)
                        return et

                    def denom_av(h, et):
                        # softmax denominator: accumulate the all-ones matmul
                        # over the 8 key sub-tiles -> partition-reduction AND
                        # broadcast in one shot (also keeps PE warm here)
                        pr = rlp.tile([P, NX], F32, tag="rl", name=f"pr{h}")
                        for kt in range(KO):
                            for qc in range(NQC):
                                qs = slice(qc * QC, (qc + 1) * QC)
                                nc.tensor.matmul(
                                    pr[:, qs], lhsT=ones_bf,
                                    rhs=et[kt][:, qs],
                                    start=(kt == 0), stop=(kt == KO - 1))
                        rc = rp.tile([P, NX], F32, tag="rc", name=f"rc{h}")
                        nc.vector.reciprocal_approx_fast(rc, pr)
                        # attnT_h[d, q] = sum_k V[k, d_h] expT[k, q]; then
                        # normalize by the softmax denom and add the Q residual
                        pa = avp.tile([P, NX], F32, tag="av", name=f"pa{h}")
                        for kt in range(KO):
                            for qc in range(NQC):
                                qs = slice(qc * QC, (qc + 1) * QC)
                                nc.tensor.matmul(
                                    pa[:, qs],
                                    lhsT=vm[:, kt, h * P:(h + 1) * P],
                                    rhs=et[kt][:, qs],
                                    start=(kt == 0), stop=(kt == KO - 1))
                        nc.vector.tensor_mul(_r(zt[:, h, :]), pa, rc)
                        nc.vector.tensor_add(_r(zt[:, h, :]), zt[:, h, :],
                                             qt[:, h, :])

                    # software pipeline: head h's logits/exp (PE+ACT) run while
                    # head h-1's denominator+AV (PE) wait on h-1's exp -> PE
                    # never idles long enough for HAM to re-throttle
                    prev = None
                    for h in range(H):
                        et = logits_exp(h)
                        if prev is not None:
                            denom_av(h - 1, prev)
                        prev = et
                    denom_av(H - 1, prev)

                # ---- LayerNorm over the model dim (partition direction) ----
                def layernorm(in_sb, sqp, stp, spp, emit_out):
                    for qc in range(NQC):
                        qs = slice(qc * QC, (qc + 1) * QC)
                        pmu = spp.tile([P, QC], F32, tag="pmu", name=f"pmu{qc}")
                        ps2 = spp.tile([P, QC], F32, tag="ps2", name=f"ps2{qc}")
                        for do in range(KO):
                            nc.tensor.matmul(pmu, lhsT=_r(ones128),
                                             rhs=_r(in_sb[:, do, qs]),
                                             start=(do == 0), stop=(do == KO - 1))
                        for do in range(KO):
                            sq = sqp.tile([P, QC], F32, tag="sq", name=f"sq{qc}{do}")
                            nc.vector.tensor_mul(_r(sq), in_sb[:, do, qs],
                                                 in_sb[:, do, qs])
                            nc.tensor.matmul(ps2, lhsT=_r(ones128), rhs=_r(sq),
                                             start=(do == 0), stop=(do == KO - 1))
                        mu = stp.tile([P, QC], F32, tag="mu", name=f"mu{qc}")
                        nc.vector.tensor_scalar_mul(mu, pmu, 1.0 / DIM)
                        msq = stp.tile([P, QC], F32, tag="msq", name=f"msq{qc}")
                        nc.vector.tensor_mul(msq, mu, mu)
                        sd = stp.tile([P, QC], F32, tag="sd", name=f"sd{qc}")
                        nc.vector.scalar_tensor_tensor(
                            sd, ps2, 1.0 / DIM, msq,
                            op0=ALU.mult, op1=ALU.subtract)
                        nc.scalar.activation(sd, sd, AF.Sqrt, bias=eps_sb, scale=1.0)
                        rsig = stp.tile([P, QC], F32, tag="rsig", name=f"rsig{qc}")
                        nc.vector.reciprocal_approx_fast(rsig, sd)
                        mrs = stp.tile([P, QC], F32, tag="mrs", name=f"mrs{qc}")
                        nc.vector.tensor_mul(mrs, mu, rsig)
                        for do in range(KO):
                            t = sqp.tile([P, QC], F32, tag="t", name=f"t{qc}{do}")
                            nc.vector.tensor_mul(t, in_sb[:, do, qs], rsig)
                            nc.vector.tensor_sub(t, t, mrs)
                            emit_out(do, qs, t)

                # LN1 -> o1t (feature-major)
                with tc.tile_pool(name="sq1", bufs=3) as sqp1, \
                     tc.tile_pool(name="st1", bufs=2) as stp1, \
                     tc.tile_pool(name="sp1", bufs=2, space="PSUM") as spp1:
                    o1t = actp.tile([P, KO, NX], F32, tag="big", name="o1t")

                    def emit_o1(do, qs, t):
                        nc.vector.tensor_scalar(
                            _r(o1t[:, do, qs]), t,
                            scalar1=g1_sb[:, do:do + 1],
                            scalar2=b1_sb[:, do:do + 1],
                            op0=ALU.mult, op1=ALU.add)

                    layernorm(zt, sqp1, stp1, spp1, emit_o1)

            # ================= Phase 3: output proj + LN2 =================
            with tc.tile_pool(name="w3", bufs=2) as wp3, \
                 tc.tile_pool(name="sq2", bufs=4) as sqp2, \
                 tc.tile_pool(name="st2", bufs=2) as stp2, \
                 tc.tile_pool(name="out", bufs=4) as outp, \
                 tc.tile_pool(name="gp3", bufs=4, space="PSUM") as pp3, \
                 tc.tile_pool(name="sp2", bufs=2, space="PSUM") as spp2:
                z2t = actp.tile([P, KO, NX], F32, tag="big", name="z2t")
                # HT[n, q] = sum_d Wo[d, n] O1T[d, q];  z2 = o1 + relu(H + bo)
                for ng in range(2):
                    wt = wp3.tile([P, KO, QC], F32, tag="w", name=f"w_o{ng}")
                    for k in range(KO):
                        nc.sync.dma_start(_r(wt[:, k, :]),
                                          _r(wo3[:, k, ng * QC:(ng + 1) * QC]))
                    for qc in range(NQC):
                        qs = slice(qc * QC, (qc + 1) * QC)
                        for n4 in range(4):
                            ps = pp3.tile([P, QC], F32, tag="ps",
                                          name=f"ps_o{ng}{qc}{n4}")
                            for k in range(KO):
                                nc.tensor.matmul(
                                    ps,
                                    lhsT=_r(wt[:, k, n4 * P:(n4 + 1) * P]),
                                    rhs=_r(o1t[:, k, qs]),
                                    start=(k == 0), stop=(k == KO - 1))
                            no = ng * 4 + n4
                            ht = sqp2.tile([P, QC], F32, tag="ht",
                                           name=f"ht{ng}{qc}{n4}")
                            nc.scalar.activation(ht, ps, AF.Relu,
                                                 bias=bo_sb[:, no:no + 1], scale=1.0)
                            nc.vector.tensor_add(_r(z2t[:, no, qs]), ht,
                                                 o1t[:, no, qs])

                def emit_o2(do, qs, t):
                    o = outp.tile([P, QC], F32, tag="o", name=f"o{do}")
                    nc.vector.tensor_scalar(
                        o, t,
                        scalar1=g2_sb[:, do:do + 1],
                        scalar2=b2_sb[:, do:do + 1],
                        op0=ALU.mult, op1=ALU.add)
                    nc.sync.dma_start(ot3[:, do, qs], o)

                layernorm(z2t, sqp2, stp2, spp2, emit_o2)

    nc.compile()
    return nc


_CACHE = {}


def _get_nc():
    if "nc" not in _CACHE:
        _CACHE["nc"] = _build()
    return _CACHE["nc"]


def make_in_maps(X, Y, mask, Wq, bq, Wk, bk, Wv, bv, Wo, bo, g1, b1, g2, b2):
    f = lambda a: np.ascontiguousarray(np.asarray(a, dtype=np.float32))
    shared = {
        "Wq": f(Wq), "Wk": f(Wk), "Wv": f(Wv), "Wo": f(Wo),
        "bq": f(bq), "bk": f(bk), "bv": f(bv), "bo": f(bo),
        "g1": f(g1), "b1": f(b1), "g2": f(g2), "b2": f(b2),
    }
    X = np.asarray(X, dtype=np.float32)
    Y = np.asarray(Y, dtype=np.float32)
    mask = np.asarray(mask)
    in_maps = []
    for b in range(8):
        mb = np.where(mask[b], np.float32(-1e4), np.float32(0.0)).astype(np.float32)
        in_maps.append({
            "XT": np.ascontiguousarray(X[b].T),
            "YT": np.ascontiguousarray(Y[b].T),
            "MB": mb,
            **shared,
        })
    return in_maps


def kernel(X, Y, mask, Wq, bq, Wk, bk, Wv, bv, Wo, bo, g1, b1, g2, b2,
           _trace=False):
    nc = _get_nc()
    in_maps = make_in_maps(X, Y, mask, Wq, bq, Wk, bk, Wv, bv, Wo, bo,
                           g1, b1, g2, b2)
    res = run_bass_kernel_spmd(nc, in_maps, core_ids=list(range(8)),
                               trace=_trace)
    out = np.stack([np.ascontiguousarray(res.results[b]["OT"].T)
                    for b in range(8)]).astype(np.float32)
    if _trace:
        return out, res
    return out



# revision 13
# speedup vs baseline: 1.4553x; 1.4553x over previous
"""Trainium2 Bass kernel for a masked-attention block (MAB).

Computation (per batch element):
    Q = X@Wq + bq ; K = Y@Wk + bk ; V = Y@Wv + bv
    logits = per-head Qh@Kh^T / 32, masked keys -> -inf, softmax over keys
    attn   = A @ Vh (concat heads)
    O1 = LN(Q + attn; g1,b1)
    O  = LN(O1 + relu(O1@Wo + bo); g2,b2)

Sharding: pure data-parallel, one batch element per NeuronCore (B=8 = 8 cores).

On-device dataflow is "feature-major": activations live in SBUF transposed
([model_dim -> 8x128 partitions, token -> free]).  With weights in natural
layout every matmul chains without any transposes.  All matmul operands are
bf16 (PE rate is identical to fp32r, but: half the DMA bytes, FWL-accelerated
LDWEIGHTS, and 2x packed DVE ops); PSUM accumulation stays fp32.

Schedule (engines run their queues in program order; this ordering is the
software pipeline):
  pre-loop : V proj (natural layout), Q proj        [PE; ACT does epilogues]
  loop h   : denom tree for h-1 [DVE], K proj h [PE, DVE epi], logits h [PE],
             exp h [ACT], AV h-1 [PE], attn epilogue h-1 [DVE]
  tail     : LN1 -> O proj -> LN2 per 512-token half, pipelined across
             PE (stats matmuls, proj) / DVE (elementwise) / ACT (affine)

The softmax denominator is a partition-dim reduction done as a bf16 pairwise
tree on DVE (frees the PE of ~65k ones-matmul columns); LayerNorm stats stay
as all-ones stationary matmuls (cheap, and they broadcast for free).

The host transposes X/Y on the way in and the output on the way out, converts
everything the matmuls touch to bf16, and turns the bool mask into an
additive f32 bias (0 / -1e4) consumed by the exp activation.
"""

import math
import numpy as np
from contextlib import ExitStack

import ml_dtypes

import concourse.bass as bass
import concourse.mybir as mybir
import concourse.tile as tile
from concourse import bacc
from concourse.bass_utils import run_bass_kernel_spmd

P = 128
NX = 1024
NY = 1024
DIM = 1024
H = 8
KO = DIM // P          # 8 partition sub-tiles of the model dim
QC = 512               # moving-operand chunk
NQC = NX // QC         # 2
F32 = mybir.dt.float32
BF16 = mybir.dt.bfloat16
AF = mybir.ActivationFunctionType
ALU = mybir.AluOpType
SCALE = 1.0 / 32.0     # 1/sqrt(DIM)
EPS = 1e-5
DEBUG = False          # adds intermediate-tensor DRAM dumps (debugging only)


def _build():
    nc = bacc.Bacc("TRN2", target_bir_lowering=False, debug=False,
                   enable_asserts=False)

    # ---- DRAM I/O (per-core shapes) ----
    XT = nc.dram_tensor("XT", [DIM, NX], BF16, kind="ExternalInput").ap()
    YT = nc.dram_tensor("YT", [DIM, NY], BF16, kind="ExternalInput").ap()
    MB = nc.dram_tensor("MB", [NY], F32, kind="ExternalInput").ap()
    Wd = {}
    for w in ("Wq", "Wk", "Wv", "Wo"):
        Wd[w] = nc.dram_tensor(w, [DIM, DIM], BF16, kind="ExternalInput").ap()
    BV = nc.dram_tensor("bv", [DIM], BF16, kind="ExternalInput").ap()
    Vecs = {}
    for vname in ("bq", "bk", "bo", "g1", "b1", "g2", "b2"):
        Vecs[vname] = nc.dram_tensor(vname, [DIM], F32, kind="ExternalInput").ap()
    OT = nc.dram_tensor("OT", [DIM, NX], BF16, kind="ExternalOutput").ap()

    xt3 = XT.rearrange("(ko p) q -> p ko q", p=P)
    yt3 = YT.rearrange("(ko p) q -> p ko q", p=P)
    wq3 = Wd["Wq"].rearrange("(ko p) d -> p ko d", p=P)
    wk3 = Wd["Wk"].rearrange("(ko p) d -> p ko d", p=P)
    wv3 = Wd["Wv"].rearrange("(ko p) d -> p ko d", p=P)
    wo3 = Wd["Wo"].rearrange("(ko p) d -> p ko d", p=P)
    ot3 = OT.rearrange("(do p) q -> p do q", p=P)

    dbg = {}
    if DEBUG:
        for nm, shp, dt in [("d_qt", [P, KO, NX], BF16),
                            ("d_ktm0", [P, NY], BF16),
                            ("d_vm", [P, KO, DIM], BF16),
                            ("d_et00", [P, NX], BF16),
                            ("d_rc0", [P, NX], F32),
                            ("d_zt", [P, KO, NX], BF16),
                            ("d_o1t", [P, KO, NX], BF16),
                            ("d_z2t", [P, KO, NX], BF16)]:
            dbg[nm] = nc.dram_tensor(nm, shp, dt, kind="ExternalOutput").ap()

    with tile.TileContext(nc) as tc:
        with ExitStack() as octx:
            const = octx.enter_context(tc.tile_pool(name="const", bufs=1))
            persist = octx.enter_context(tc.tile_pool(name="persist", bufs=1))
            actp = octx.enter_context(tc.tile_pool(name="act", bufs=3))

            # ---- constants (issue the small DMAs first on the sync queue) ----
            ones_bf = const.tile([P, P], BF16, tag="onesbf", name="ones_bf")
            nc.vector.memset(ones_bf, 1.0)
            eps_sb = const.tile([P, 1], F32, tag="eps", name="eps_sb")
            nc.vector.memset(eps_sb, EPS)

            def vec_pko(name):
                t = const.tile([P, KO], F32, tag=f"v_{name}", name=f"{name}_sb")
                nc.sync.dma_start(t, Vecs[name].rearrange("(ko p) -> p ko", p=P))
                return t

            mb_sb = const.tile([P, KO], F32, tag="v_mb", name="mb_sb")
            nc.sync.dma_start(mb_sb, MB.rearrange("(ko p) -> p ko", p=P))
            bq_sb = vec_pko("bq")
            bk_sb = vec_pko("bk")
            bo_sb = vec_pko("bo")
            g1_sb = vec_pko("g1")
            b1_sb = vec_pko("b1")
            g2_sb = vec_pko("g2")
            b2_sb = vec_pko("b2")
            bv_sb = const.tile([1, DIM], BF16, tag="v_bv", name="bv_sb")
            nc.sync.dma_start(bv_sb, BV.rearrange("(one n) -> one n", one=1))

            # ---- persistent tensors + prioritized input DMA order ----
            # priority: yt (V proj gate) > wv > xt, wq (Q proj) > wk > wo
            yt = persist.tile([P, KO, NY], BF16, tag="yt", name="yt")
            nc.sync.dma_start(yt, yt3)
            vm = persist.tile([P, KO, DIM], BF16, tag="vm", name="vm")
            wo_sb = persist.tile([P, KO, DIM], BF16, tag="wo", name="wo_sb")
            wkp = octx.enter_context(tc.tile_pool(name="wkp", bufs=2))
            ktmp = octx.enter_context(tc.tile_pool(name="ktmp", bufs=2))

            # big feature-major activation tiles (rotate through 3 slots)
            qt = actp.tile([P, KO, NX], BF16, tag="big", name="qt")

            # ================= Phase 1: V and Q projections =================
            with tc.tile_pool(name="io", bufs=1) as iop, \
                 tc.tile_pool(name="gp1", bufs=8, space="PSUM") as pp:
                wv_sb = iop.tile([P, KO, DIM], BF16, tag="wv", name="wv_sb")
                nc.sync.dma_start(wv_sb, wv3)
                xt = iop.tile([P, KO, NX], BF16, tag="xt", name="xt")
                nc.sync.dma_start(xt, xt3)
                wq_sb = iop.tile([P, KO, DIM], BF16, tag="wq", name="wq_sb")
                nc.sync.dma_start(wq_sb, wq3)
                wk_tiles = []
                for h in range(2):
                    wkt = wkp.tile([P, KO, P], BF16, tag="wk", name=f"wk{h}")
                    nc.sync.dma_start(wkt, wk3[:, :, h * P:(h + 1) * P])
                    wk_tiles.append(wkt)
                nc.sync.dma_start(wo_sb, wo3)

                # V in natural (token-major) layout: V[y, n] = sum_k Y[y,k] Wv[k,n]
                # bias bv folded with a K=1 ones matmul; ACT copies psum -> vm
                for yo in range(KO):
                    pss = [pp.tile([P, QC], F32, tag="ps", name=f"ps_v{yo}{ng}")
                           for ng in range(2)]
                    for k in range(KO):
                        for ng in range(2):
                            ns = slice(ng * QC, (ng + 1) * QC)
                            nc.tensor.matmul(
                                pss[ng],
                                lhsT=yt[:, k, yo * P:(yo + 1) * P],
                                rhs=wv_sb[:, k, ns],
                                start=(k == 0), stop=False)
                    for ng in range(2):
                        ns = slice(ng * QC, (ng + 1) * QC)
                        nc.tensor.matmul(
                            pss[ng], lhsT=ones_bf[0:1, :], rhs=bv_sb[:, ns],
                            start=False, stop=True)
                        nc.scalar.copy(vm[:, yo, ns], pss[ng])

                # Q feature-major: qt[p, do, q] = sum_k Wq[k, d] xt[k, q] + bq
                for do in range(KO):
                    for qc in range(NQC):
                        qs = slice(qc * QC, (qc + 1) * QC)
                        ps = pp.tile([P, QC], F32, tag="ps", name=f"ps_q{do}{qc}")
                        for k in range(KO):
                            nc.tensor.matmul(
                                ps,
                                lhsT=wq_sb[:, k, do * P:(do + 1) * P],
                                rhs=xt[:, k, qs],
                                start=(k == 0), stop=(k == KO - 1))
                        nc.scalar.activation(
                            qt[:, do, qs], ps, AF.Identity,
                            bias=bq_sb[:, do:do + 1], scale=1.0)

            if DEBUG:
                nc.sync.dma_start(dbg["d_qt"], qt)
                nc.sync.dma_start(dbg["d_vm"], vm)

            # ================= Phase 2: K proj + attention (pipelined) ======
            zt = actp.tile([P, KO, NX], BF16, tag="big", name="zt")

            with tc.tile_pool(name="kq", bufs=2, space="PSUM") as kqp, \
                 tc.tile_pool(name="lgp", bufs=2, space="PSUM") as lgp, \
                 tc.tile_pool(name="avp", bufs=1, space="PSUM") as avp, \
                 tc.tile_pool(name="exp", bufs=17) as ep, \
                 tc.tile_pool(name="prs", bufs=7) as prp, \
                 tc.tile_pool(name="den", bufs=2) as dnp, \
                 tc.tile_pool(name="rcp", bufs=2) as rcp:

                def kproj(h):
                    # K slab h: ktm_h[p, y] = sum_k Wk[k, h*128+p] yt[k, y] + bk
                    ktm_h = ktmp.tile([P, NY], BF16, tag="ktm", name=f"ktm{h}")
                    for qc in range(NQC):
                        qs = slice(qc * QC, (qc + 1) * QC)
                        pk = kqp.tile([P, QC], F32, tag="pk", name=f"pk{h}{qc}")
                        for k in range(KO):
                            nc.tensor.matmul(
                                pk, lhsT=wk_tiles[h][:, k, :], rhs=yt[:, k, qs],
                                start=(k == 0), stop=(k == KO - 1))
                        nc.vector.tensor_scalar(
                            ktm_h[:, qs], pk, bk_sb[:, h:h + 1], None,
                            op0=ALU.add)
                    if h + 2 < H:
                        # prefetch the h+2 weight chunk; issued after this
                        # head's matmuls so the 2-deep pool rotation can't
                        # clobber a chunk that still has unissued readers
                        wkt = wkp.tile([P, KO, P], BF16, tag="wk", name=f"wk{h+2}")
                        nc.sync.dma_start(wkt, wk3[:, :, (h + 2) * P:(h + 3) * P])
                        wk_tiles.append(wkt)
                    return ktm_h

                def logits_exp(h, ktm_h):
                    # logitsT[k, q] = sum_d KT_h[d, k] QT_h[d, q]; exp with
                    # mask bias per key (partition) and 1/32 scale
                    et = [ep.tile([P, NX], BF16, tag="exp", name=f"et{h}_{k}")
                          for k in range(KO)]
                    for kt in range(KO):
                        pl = lgp.tile([P, NX], F32, tag="lg", name=f"pl{h}{kt}")
                        for qc in range(NQC):
                            qs = slice(qc * QC, (qc + 1) * QC)
                            nc.tensor.matmul(
                                pl[:, qs],
                                lhsT=ktm_h[:, kt * P:(kt + 1) * P],
                                rhs=qt[:, h, qs],
                                start=True, stop=True)
                        nc.scalar.activation(
                            et[kt], pl, AF.Exp,
                            bias=mb_sb[:, kt:kt + 1], scale=SCALE)
                    return et

                def denom_tree(h, et):
                    # softmax denominator, part 1 (DVE only): cross-tile
                    # pairwise tree (bf16 2x) collapses the 8 key-slab tiles
                    # into one; issued at iteration start to fill the DVE
                    # bubble while the PE runs this head's K projection
                    l1 = [prp.tile([P, NX], BF16, tag="pr", name=f"l1_{h}{i}")
                          for i in range(4)]
                    for i in range(4):
                        nc.vector.tensor_add(l1[i], et[2 * i], et[2 * i + 1])
                    l2 = [prp.tile([P, NX], BF16, tag="pr", name=f"l2_{h}{i}")
                          for i in range(2)]
                    for i in range(2):
                        nc.vector.tensor_add(l2[i], l1[2 * i], l1[2 * i + 1])
                    den = dnp.tile([P, NX], BF16, tag="dn", name=f"den{h}")
                    nc.vector.tensor_add(den, l2[0], l2[1])
                    return den

                def denom_finish(h, den):
                    # part 2: a small all-ones matmul does the in-slab
                    # 128-partition reduction (and broadcasts for free)
                    rc = rcp.tile([P, NX], F32, tag="rc", name=f"rc{h}")
                    for qc in range(NQC):
                        qs = slice(qc * QC, (qc + 1) * QC)
                        pr = kqp.tile([P, QC], F32, tag="pk", name=f"pr{h}{qc}")
                        nc.tensor.matmul(pr, lhsT=ones_bf, rhs=den[:, qs],
                                         start=True, stop=True)
                        nc.vector.reciprocal_approx_fast(rc[:, qs], pr)
                    return rc

                def av_epi(h, et, rc):
                    # attnT_h[d, q] = sum_k V[k, d_h] expT[k, q]; normalize by
                    # the softmax denominator and add the Q residual
                    pa = avp.tile([P, NX], F32, tag="av", name=f"pa{h}")
                    for kt in range(KO):
                        for qc in range(NQC):
                            qs = slice(qc * QC, (qc + 1) * QC)
                            nc.tensor.matmul(
                                pa[:, qs],
                                lhsT=vm[:, kt, h * P:(h + 1) * P],
                                rhs=et[kt][:, qs],
                                start=(kt == 0), stop=(kt == KO - 1))
                    nc.vector.tensor_mul(zt[:, h, :], pa, rc)
                    nc.vector.tensor_add(zt[:, h, :], zt[:, h, :], qt[:, h, :])

                # software pipeline: head h's K proj / logits / exp overlap
                # head h-1's denominator (DVE tree + tiny ones-matmul) and
                # AV (PE).  Issue order is chosen per engine queue: the DVE
                # tree first (ready at iteration start), the PE denominator
                # matmuls after the logits matmuls (so the in-order PE queue
                # never waits on the tree).
                prev = None
                den_prev = None
                for h in range(H):
                    if prev is not None:
                        den_prev = denom_tree(h - 1, prev)
                    ktm_h = kproj(h)
                    et = logits_exp(h, ktm_h)
                    if DEBUG and h == 0:
                        nc.sync.dma_start(dbg["d_ktm0"], ktm_h)
                        nc.sync.dma_start(dbg["d_et00"], et[0])
                    if prev is not None:
                        rc_prev = denom_finish(h - 1, den_prev)
                        if DEBUG and h == 1:
                            nc.sync.dma_start(dbg["d_rc0"], rc_prev)
                        av_epi(h - 1, prev, rc_prev)
                    prev = et
                den_prev = denom_tree(H - 1, prev)
                rc_prev = denom_finish(H - 1, den_prev)
                av_epi(H - 1, prev, rc_prev)
                if DEBUG:
                    nc.sync.dma_start(dbg["d_zt"], zt)

            # ================= Phase 3: LN1 -> O proj + LN2 =================
            # LayerNorm over the model dim (partition direction): stats via
            # all-ones stationary matmuls (free broadcast), elementwise on
            # DVE, gamma/beta affine on ACT.
            with tc.tile_pool(name="sqp", bufs=2) as sqp, \
                 tc.tile_pool(name="stp", bufs=2) as stp, \
                 tc.tile_pool(name="out", bufs=4) as outp, \
                 tc.tile_pool(name="spp", bufs=2, space="PSUM") as spp, \
                 tc.tile_pool(name="gp3", bufs=4, space="PSUM") as pp3:

                def layernorm_qc(in_sb, qc, g_sb, b_sb, emit_out, lbl):
                    qs = slice(qc * QC, (qc + 1) * QC)
                    pmu = spp.tile([P, QC], F32, tag="pmu", name=f"pmu{lbl}{qc}")
                    for do in range(KO):
                        nc.tensor.matmul(pmu, lhsT=ones_bf,
                                         rhs=in_sb[:, do, qs],
                                         start=(do == 0), stop=(do == KO - 1))
                    sq = sqp.tile([P, KO, QC], BF16, tag="sq", name=f"sq{lbl}{qc}")
                    nc.vector.tensor_mul(sq, in_sb[:, :, qs], in_sb[:, :, qs])
                    ps2 = spp.tile([P, QC], F32, tag="ps2", name=f"ps2{lbl}{qc}")
                    for do in range(KO):
                        nc.tensor.matmul(ps2, lhsT=ones_bf, rhs=sq[:, do, :],
                                         start=(do == 0), stop=(do == KO - 1))
                    mu = stp.tile([P, QC], F32, tag="mu", name=f"mu{lbl}{qc}")
                    nc.vector.tensor_scalar_mul(mu, pmu, 1.0 / DIM)
                    msq = stp.tile([P, QC], F32, tag="msq", name=f"msq{lbl}{qc}")
                    nc.vector.tensor_mul(msq, mu, mu)
                    sd = stp.tile([P, QC], F32, tag="sd", name=f"sd{lbl}{qc}")
                    nc.vector.scalar_tensor_tensor(
                        sd, ps2, 1.0 / DIM, msq,
                        op0=ALU.mult, op1=ALU.subtract)
                    nc.scalar.activation(sd, sd, AF.Sqrt, bias=eps_sb, scale=1.0)
                    rsig = stp.tile([P, QC], F32, tag="rsig", name=f"rsig{lbl}{qc}")
                    nc.vector.reciprocal_approx_fast(rsig, sd)
                    mub = stp.tile([P, QC], BF16, tag="mub", name=f"mub{lbl}{qc}")
                    nc.vector.tensor_copy(mub, mu)
                    rsb = stp.tile([P, QC], BF16, tag="rsb", name=f"rsb{lbl}{qc}")
                    nc.vector.tensor_copy(rsb, rsig)
                    # t = (x - mu) * rsig over the whole [128, 8, 512] block
                    t = sqp.tile([P, KO, QC], BF16, tag="t", name=f"t{lbl}{qc}")
                    mu_b = mub.unsqueeze(1).broadcast_to([P, KO, QC])
                    rs_b = rsb.unsqueeze(1).broadcast_to([P, KO, QC])
                    nc.vector.tensor_sub(t, in_sb[:, :, qs], mu_b)
                    nc.vector.tensor_mul(t, t, rs_b)
                    for do in range(KO):
                        # out = t * g + b  (per-partition affine on ACT)
                        emit_out(do, qs, t[:, do, :],
                                 g_sb[:, do:do + 1], b_sb[:, do:do + 1])

                o1t = actp.tile([P, KO, NX], BF16, tag="big", name="o1t")

                def emit_o1(do, qs, t, g_col, b_col):
                    nc.scalar.activation(o1t[:, do, qs], t, AF.Identity,
                                         bias=b_col, scale=g_col)

                z2t = actp.tile([P, KO, NX], BF16, tag="big", name="z2t")

                def oproj_qc(qc):
                    # HT[n, q] = sum_d Wo[d, n] O1T[d, q]; z2 = o1+relu(H+bo)
                    qs = slice(qc * QC, (qc + 1) * QC)
                    for no in range(KO):
                        po = pp3.tile([P, QC], F32, tag="po", name=f"po{no}{qc}")
                        for k in range(KO):
                            nc.tensor.matmul(
                                po,
                                lhsT=wo_sb[:, k, no * P:(no + 1) * P],
                                rhs=o1t[:, k, qs],
                                start=(k == 0), stop=(k == KO - 1))
                        ht = outp.tile([P, QC], BF16, tag="ht", name=f"ht{no}{qc}")
                        nc.scalar.activation(ht, po, AF.Relu,
                                             bias=bo_sb[:, no:no + 1], scale=1.0)
                        nc.vector.tensor_add(z2t[:, no, qs], ht, o1t[:, no, qs])

                def emit_o2(do, qs, t, g_col, b_col):
                    o = outp.tile([P, QC], BF16, tag="o", name=f"o{do}")
                    nc.scalar.activation(o, t, AF.Identity,
                                         bias=b_col, scale=g_col)
                    nc.sync.dma_start(ot3[:, do, qs], o)

                for qc in range(NQC):
                    layernorm_qc(zt, qc, g1_sb, b1_sb, emit_o1, "a")
                if DEBUG:
                    nc.sync.dma_start(dbg["d_o1t"], o1t)
                for qc in range(NQC):
                    oproj_qc(qc)
                if DEBUG:
                    nc.sync.dma_start(dbg["d_z2t"], z2t)
                for qc in range(NQC):
                    layernorm_qc(z2t, qc, g2_sb, b2_sb, emit_o2, "b")

    nc.compile()
    return nc


_CACHE = {}


def _get_nc():
    if "nc" not in _CACHE:
        _CACHE["nc"] = _build()
    return _CACHE["nc"]


def make_in_maps(X, Y, mask, Wq, bq, Wk, bk, Wv, bv, Wo, bo, g1, b1, g2, b2):
    bf = lambda a: np.ascontiguousarray(
        np.asarray(a, dtype=np.float32).astype(ml_dtypes.bfloat16))
    f = lambda a: np.ascontiguousarray(np.asarray(a, dtype=np.float32))
    shared = {
        "Wq": bf(Wq), "Wk": bf(Wk), "Wv": bf(Wv), "Wo": bf(Wo),
        "bq": f(bq), "bk": f(bk), "bv": bf(bv), "bo": f(bo),
        "g1": f(g1), "b1": f(b1), "g2": f(g2), "b2": f(b2),
    }
    X = np.asarray(X, dtype=np.float32)
    Y = np.asarray(Y, dtype=np.float32)
    mask = np.asarray(mask)
    in_maps = []
    for b in range(8):
        mb = np.where(mask[b], np.float32(-1e4), np.float32(0.0)).astype(np.float32)
        in_maps.append({
            "XT": bf(X[b].T),
            "YT": bf(Y[b].T),
            "MB": mb,
            **shared,
        })
    return in_maps


def kernel(X, Y, mask, Wq, bq, Wk, bk, Wv, bv, Wo, bo, g1, b1, g2, b2,
           _trace=False):
    nc = _get_nc()
    in_maps = make_in_maps(X, Y, mask, Wq, bq, Wk, bk, Wv, bv, Wo, bo,
                           g1, b1, g2, b2)
    res = run_bass_kernel_spmd(nc, in_maps, core_ids=list(range(8)),
                               trace=_trace)
    out = np.stack([np.asarray(res.results[b]["OT"]).astype(np.float32).T
                    for b in range(8)])
    out = np.ascontiguousarray(out)
    if _trace:
        return out, res
    return out


# revision 20
# speedup vs baseline: 1.5094x; 1.0372x over previous
"""Trainium2 Bass kernel for a masked-attention block (MAB).

Computation (per batch element):
    Q = X@Wq + bq ; K = Y@Wk + bk ; V = Y@Wv + bv
    logits = per-head Qh@Kh^T / 32, masked keys -> -inf, softmax over keys
    attn   = A @ Vh (concat heads)
    O1 = LN(Q + attn; g1,b1)
    O  = LN(O1 + relu(O1@Wo + bo); g2,b2)

Sharding: pure data-parallel, one batch element per NeuronCore (B=8 = 8 cores).

On-device dataflow is "feature-major": activations live in SBUF transposed
([model_dim -> 8x128 partitions, token -> free]).  With weights in natural
layout every matmul chains without any transposes.  All matmul operands are
bf16 (PE rate is identical to fp32r, but: half the DMA bytes, FWL-accelerated
LDWEIGHTS, and 2x packed DVE ops); PSUM accumulation stays fp32.

Schedule (engines run their queues in program order; this ordering is the
software pipeline):
  pre-loop : V proj (natural layout), Q proj        [PE; ACT does epilogues]
  loop h   : denom tree for h-1 [DVE], K proj h [PE, DVE epi], logits h [PE],
             exp h [ACT], AV h-1 [PE], attn epilogue h-1 [DVE]
  tail     : LN1 -> O proj -> LN2 per 512-token half, pipelined across
             PE (stats matmuls, proj) / DVE (elementwise) / ACT (affine)

The softmax denominator is a partition-dim reduction done as a bf16 pairwise
tree on DVE (frees the PE of ~65k ones-matmul columns); LayerNorm stats stay
as all-ones stationary matmuls (cheap, and they broadcast for free).

The host transposes X/Y on the way in and the output on the way out, converts
everything the matmuls touch to bf16, and turns the bool mask into an
additive f32 bias (0 / -1e4) consumed by the exp activation.
"""

import math
import numpy as np
from contextlib import ExitStack

import ml_dtypes

import concourse.bass as bass
import concourse.mybir as mybir
import concourse.tile as tile
from concourse import bacc
from concourse.bass_utils import run_bass_kernel_spmd

P = 128
NX = 1024
NY = 1024
DIM = 1024
H = 8
KO = DIM // P          # 8 partition sub-tiles of the model dim
QC = 512               # moving-operand chunk
NQC = NX // QC         # 2
F32 = mybir.dt.float32
BF16 = mybir.dt.bfloat16
AF = mybir.ActivationFunctionType
ALU = mybir.AluOpType
SCALE = 1.0 / 32.0     # 1/sqrt(DIM)
EPS = 1e-5
DEBUG = False          # adds intermediate-tensor DRAM dumps (debugging only)


def _build():
    nc = bacc.Bacc("TRN2", target_bir_lowering=False, debug=False,
                   enable_asserts=False)

    # ---- DRAM I/O (per-core shapes) ----
    XT = nc.dram_tensor("XT", [DIM, NX], BF16, kind="ExternalInput").ap()
    YT = nc.dram_tensor("YT", [DIM, NY], BF16, kind="ExternalInput").ap()
    MB = nc.dram_tensor("MB", [NY], F32, kind="ExternalInput").ap()
    Wd = {}
    for w in ("Wq", "Wk", "Wv", "Wo"):
        Wd[w] = nc.dram_tensor(w, [DIM, DIM], BF16, kind="ExternalInput").ap()
    Vecs = {}
    for vname in ("bq", "bk", "bv", "bo", "g1", "b1", "g2", "b2"):
        Vecs[vname] = nc.dram_tensor(vname, [DIM], F32, kind="ExternalInput").ap()
    OT = nc.dram_tensor("OT", [DIM, NX], BF16, kind="ExternalOutput").ap()

    xt3 = XT.rearrange("(ko p) q -> p ko q", p=P)
    yt3 = YT.rearrange("(ko p) q -> p ko q", p=P)
    wq3 = Wd["Wq"].rearrange("(ko p) d -> p ko d", p=P)
    wk3 = Wd["Wk"].rearrange("(ko p) d -> p ko d", p=P)
    wv3 = Wd["Wv"].rearrange("(ko p) d -> p ko d", p=P)
    wo3 = Wd["Wo"].rearrange("(ko p) d -> p ko d", p=P)
    ot3 = OT.rearrange("(do p) q -> p do q", p=P)

    dbg = {}
    if DEBUG:
        for nm, shp, dt in [("d_qt", [P, KO, NX], BF16),
                            ("d_ktm0", [P, NY], BF16),
                            ("d_vm", [P, KO, DIM], BF16),
                            ("d_et00", [P, NX], BF16),
                            ("d_rc0", [P, NX], F32),
                            ("d_zt", [P, KO, NX], BF16),
                            ("d_o1t", [P, KO, NX], BF16),
                            ("d_z2t", [P, KO, NX], BF16)]:
            dbg[nm] = nc.dram_tensor(nm, shp, dt, kind="ExternalOutput").ap()

    with tile.TileContext(nc) as tc:
        with ExitStack() as octx:
            const = octx.enter_context(tc.tile_pool(name="const", bufs=1))
            persist = octx.enter_context(tc.tile_pool(name="persist", bufs=1))
            actp = octx.enter_context(tc.tile_pool(name="act", bufs=3))

            # ---- constants (issue the small DMAs first on the sync queue) ----
            ones_bf = const.tile([P, P], BF16, tag="onesbf", name="ones_bf")
            nc.vector.memset(ones_bf, 1.0)
            eps_sb = const.tile([P, 1], F32, tag="eps", name="eps_sb")
            nc.vector.memset(eps_sb, EPS)

            def vec_pko(name):
                t = const.tile([P, KO], F32, tag=f"v_{name}", name=f"{name}_sb")
                nc.sync.dma_start(t, Vecs[name].rearrange("(ko p) -> p ko", p=P))
                return t

            mb_sb = const.tile([P, KO], F32, tag="v_mb", name="mb_sb")
            nc.sync.dma_start(mb_sb, MB.rearrange("(ko p) -> p ko", p=P))
            bq_sb = vec_pko("bq")
            bk_sb = vec_pko("bk")
            bv_sb = vec_pko("bv")
            bo_sb = vec_pko("bo")
            g1_sb = vec_pko("g1")
            b1_sb = vec_pko("b1")
            g2_sb = vec_pko("g2")
            b2_sb = vec_pko("b2")

            # ---- persistent tensors ----
            yt = persist.tile([P, KO, NY], BF16, tag="yt", name="yt")
            vm = persist.tile([P, KO, DIM], BF16, tag="vm", name="vm")
            wo_sb = persist.tile([P, KO, DIM], BF16, tag="wo", name="wo_sb")
            wkp = octx.enter_context(tc.tile_pool(name="wkp", bufs=2))
            ktmp = octx.enter_context(tc.tile_pool(name="ktmp", bufs=2))

            # big feature-major activation tiles (rotate through 3 slots)
            qt = actp.tile([P, KO, NX], BF16, tag="big", name="qt")

            # ================= Phase 1: V and Q projections =================
            with tc.tile_pool(name="io", bufs=1) as iop, \
                 tc.tile_pool(name="gp1", bufs=8, space="PSUM") as pp:
                wv_sb = iop.tile([P, KO, DIM], BF16, tag="wv", name="wv_sb")
                xt = iop.tile([P, KO, NX], BF16, tag="xt", name="xt")
                wq_sb = iop.tile([P, KO, DIM], BF16, tag="wq", name="wq_sb")
                # per-k chunked DMAs, interleaved by priority so the V proj
                # can start as soon as the first (yt, wv) chunk pair lands
                for k in range(KO):
                    nc.sync.dma_start(yt[:, k, :], yt3[:, k, :])
                    nc.sync.dma_start(wv_sb[:, k, :], wv3[:, k, :])
                for k in range(KO):
                    nc.sync.dma_start(xt[:, k, :], xt3[:, k, :])
                    nc.sync.dma_start(wq_sb[:, k, :], wq3[:, k, :])
                wk_tiles = []
                for h in range(2):
                    wkt = wkp.tile([P, KO, P], BF16, tag="wk", name=f"wk{h}")
                    nc.sync.dma_start(wkt, wk3[:, :, h * P:(h + 1) * P])
                    wk_tiles.append(wkt)
                nc.sync.dma_start(wo_sb, wo3)

                # V in natural (token-major) layout: V[y, n] = sum_k Y[y,k] Wv[k,n]
                # (bias bv is NOT added here: softmax rows sum to 1, so it is
                # folded into the attention epilogue instead)
                for yo in range(KO):
                    pss = [pp.tile([P, QC], F32, tag="ps", name=f"ps_v{yo}{ng}")
                           for ng in range(2)]
                    for k in range(KO):
                        for ng in range(2):
                            ns = slice(ng * QC, (ng + 1) * QC)
                            nc.tensor.matmul(
                                pss[ng],
                                lhsT=yt[:, k, yo * P:(yo + 1) * P],
                                rhs=wv_sb[:, k, ns],
                                start=(k == 0), stop=(k == KO - 1))
                    for ng in range(2):
                        ns = slice(ng * QC, (ng + 1) * QC)
                        nc.scalar.copy(vm[:, yo, ns], pss[ng])

                # Q feature-major: qt[p, do, q] = sum_k Wq[k, d] xt[k, q] + bq
                for do in range(KO):
                    for qc in range(NQC):
                        qs = slice(qc * QC, (qc + 1) * QC)
                        ps = pp.tile([P, QC], F32, tag="ps", name=f"ps_q{do}{qc}")
                        for k in range(KO):
                            nc.tensor.matmul(
                                ps,
                                lhsT=wq_sb[:, k, do * P:(do + 1) * P],
                                rhs=xt[:, k, qs],
                                start=(k == 0), stop=(k == KO - 1))
                        nc.scalar.activation(
                            qt[:, do, qs], ps, AF.Identity,
                            bias=bq_sb[:, do:do + 1], scale=1.0)

            if DEBUG:
                nc.sync.dma_start(dbg["d_qt"], qt)
                nc.sync.dma_start(dbg["d_vm"], vm)

            # ================= Phase 2: K proj + attention (pipelined) ======
            zt = actp.tile([P, KO, NX], BF16, tag="big", name="zt")

            with tc.tile_pool(name="kq", bufs=2, space="PSUM") as kqp, \
                 tc.tile_pool(name="lgp", bufs=2, space="PSUM") as lgp, \
                 tc.tile_pool(name="avp", bufs=1, space="PSUM") as avp, \
                 tc.tile_pool(name="exp", bufs=17) as ep, \
                 tc.tile_pool(name="prs", bufs=7) as prp, \
                 tc.tile_pool(name="den", bufs=2) as dnp, \
                 tc.tile_pool(name="rcp", bufs=2) as rcp:

                def kproj(h):
                    # K slab h: ktm_h[p, y] = sum_k Wk[k, h*128+p] yt[k, y] + bk
                    ktm_h = ktmp.tile([P, NY], BF16, tag="ktm", name=f"ktm{h}")
                    for qc in range(NQC):
                        qs = slice(qc * QC, (qc + 1) * QC)
                        pk = kqp.tile([P, QC], F32, tag="pk", name=f"pk{h}{qc}")
                        for k in range(KO):
                            nc.tensor.matmul(
                                pk, lhsT=wk_tiles[h][:, k, :], rhs=yt[:, k, qs],
                                start=(k == 0), stop=(k == KO - 1))
                        nc.vector.tensor_scalar(
                            ktm_h[:, qs], pk, bk_sb[:, h:h + 1], None,
                            op0=ALU.add)
                    if h + 2 < H:
                        # prefetch the h+2 weight chunk; issued after this
                        # head's matmuls so the 2-deep pool rotation can't
                        # clobber a chunk that still has unissued readers
                        wkt = wkp.tile([P, KO, P], BF16, tag="wk", name=f"wk{h+2}")
                        nc.sync.dma_start(wkt, wk3[:, :, (h + 2) * P:(h + 3) * P])
                        wk_tiles.append(wkt)
                    return ktm_h

                def logits_exp(h, ktm_h):
                    # logitsT[k, q] = sum_d KT_h[d, k] QT_h[d, q]; exp with
                    # mask bias per key (partition) and 1/32 scale
                    et = [ep.tile([P, NX], BF16, tag="exp", name=f"et{h}_{k}")
                          for k in range(KO)]
                    for kt in range(KO):
                        pl = lgp.tile([P, NX], F32, tag="lg", name=f"pl{h}{kt}")
                        for qc in range(NQC):
                            qs = slice(qc * QC, (qc + 1) * QC)
                            nc.tensor.matmul(
                                pl[:, qs],
                                lhsT=ktm_h[:, kt * P:(kt + 1) * P],
                                rhs=qt[:, h, qs],
                                start=True, stop=True)
                        nc.scalar.activation(
                            et[kt], pl, AF.Exp,
                            bias=mb_sb[:, kt:kt + 1], scale=SCALE)
                    return et

                def denom_tree(h, et):
                    # softmax denominator, part 1 (DVE only): cross-tile
                    # pairwise tree (bf16 2x) collapses the 8 key-slab tiles
                    # into one; issued at iteration start to fill the DVE
                    # bubble while the PE runs this head's K projection
                    l1 = [prp.tile([P, NX], BF16, tag="pr", name=f"l1_{h}{i}")
                          for i in range(4)]
                    for i in range(4):
                        nc.vector.tensor_add(l1[i], et[2 * i], et[2 * i + 1])
                    l2 = [prp.tile([P, NX], BF16, tag="pr", name=f"l2_{h}{i}")
                          for i in range(2)]
                    for i in range(2):
                        nc.vector.tensor_add(l2[i], l1[2 * i], l1[2 * i + 1])
                    den = dnp.tile([P, NX], BF16, tag="dn", name=f"den{h}")
                    nc.vector.tensor_add(den, l2[0], l2[1])
                    return den

                def denom_finish(h, den):
                    # part 2: a small all-ones matmul does the in-slab
                    # 128-partition reduction (and broadcasts for free)
                    rc = rcp.tile([P, NX], F32, tag="rc", name=f"rc{h}")
                    for qc in range(NQC):
                        qs = slice(qc * QC, (qc + 1) * QC)
                        pr = kqp.tile([P, QC], F32, tag="pk", name=f"pr{h}{qc}")
                        nc.tensor.matmul(pr, lhsT=ones_bf, rhs=den[:, qs],
                                         start=True, stop=True)
                        nc.vector.reciprocal_approx_fast(rc[:, qs], pr)
                    return rc

                def av_epi(h, et, rc):
                    # attnT_h[d, q] = sum_k V[k, d_h] expT[k, q]; normalize by
                    # the softmax denominator, add bv (softmax rows sum to 1,
                    # so + bv after normalize == bias inside the V proj) and
                    # the Q residual
                    pa = avp.tile([P, NX], F32, tag="av", name=f"pa{h}")
                    for kt in range(KO):
                        for qc in range(NQC):
                            qs = slice(qc * QC, (qc + 1) * QC)
                            nc.tensor.matmul(
                                pa[:, qs],
                                lhsT=vm[:, kt, h * P:(h + 1) * P],
                                rhs=et[kt][:, qs],
                                start=(kt == 0), stop=(kt == KO - 1))
                    nc.vector.tensor_mul(zt[:, h, :], pa, rc)
                    nc.vector.scalar_tensor_tensor(
                        zt[:, h, :], zt[:, h, :], bv_sb[:, h:h + 1],
                        qt[:, h, :], op0=ALU.add, op1=ALU.add)

                # software pipeline: head h's K proj / logits / exp overlap
                # head h-1's denominator (DVE tree + tiny ones-matmul) and
                # AV (PE).  Issue order is chosen per engine queue: the DVE
                # tree first (ready at iteration start), the PE denominator
                # matmuls after the logits matmuls (so the in-order PE queue
                # never waits on the tree).
                prev = None
                den_prev = None
                for h in range(H):
                    if prev is not None:
                        den_prev = denom_tree(h - 1, prev)
                    ktm_h = kproj(h)
                    et = logits_exp(h, ktm_h)
                    if DEBUG and h == 0:
                        nc.sync.dma_start(dbg["d_ktm0"], ktm_h)
                        nc.sync.dma_start(dbg["d_et00"], et[0])
                    if prev is not None:
                        rc_prev = denom_finish(h - 1, den_prev)
                        if DEBUG and h == 1:
                            nc.sync.dma_start(dbg["d_rc0"], rc_prev)
                        av_epi(h - 1, prev, rc_prev)
                    prev = et
                den_prev = denom_tree(H - 1, prev)
                rc_prev = denom_finish(H - 1, den_prev)
                av_epi(H - 1, prev, rc_prev)
                if DEBUG:
                    nc.sync.dma_start(dbg["d_zt"], zt)

            # ================= Phase 3: LN1 -> O proj + LN2 =================
            # LayerNorm over the model dim (partition direction): stats via
            # all-ones stationary matmuls (free broadcast), elementwise on
            # DVE, gamma/beta affine on ACT.
            with tc.tile_pool(name="sqp", bufs=2) as sqp, \
                 tc.tile_pool(name="stp", bufs=2) as stp, \
                 tc.tile_pool(name="out", bufs=4) as outp, \
                 tc.tile_pool(name="spp", bufs=2, space="PSUM") as spp, \
                 tc.tile_pool(name="gp3", bufs=4, space="PSUM") as pp3:

                def layernorm_chunk(in_sb, lo, hi, g_sb, b_sb, emit_out, lbl):
                    cs = hi - lo
                    qs = slice(lo, hi)
                    pmu = spp.tile([P, QC], F32, tag="pmu", name=f"pmu{lbl}{lo}")
                    for do in range(KO):
                        nc.tensor.matmul(pmu[:, :cs], lhsT=ones_bf,
                                         rhs=in_sb[:, do, qs],
                                         start=(do == 0), stop=(do == KO - 1))
                    sq = sqp.tile([P, KO, QC], BF16, tag="sq", name=f"sq{lbl}{lo}")
                    nc.vector.tensor_mul(sq[:, :, :cs], in_sb[:, :, qs],
                                         in_sb[:, :, qs])
                    ps2 = spp.tile([P, QC], F32, tag="ps2", name=f"ps2{lbl}{lo}")
                    for do in range(KO):
                        nc.tensor.matmul(ps2[:, :cs], lhsT=ones_bf,
                                         rhs=sq[:, do, :cs],
                                         start=(do == 0), stop=(do == KO - 1))
                    mu = stp.tile([P, QC], F32, tag="mu", name=f"mu{lbl}{lo}")[:, :cs]
                    nc.vector.tensor_scalar_mul(mu, pmu[:, :cs], 1.0 / DIM)
                    msq = stp.tile([P, QC], F32, tag="msq", name=f"msq{lbl}{lo}")[:, :cs]
                    nc.vector.tensor_mul(msq, mu, mu)
                    sd = stp.tile([P, QC], F32, tag="sd", name=f"sd{lbl}{lo}")[:, :cs]
                    nc.vector.scalar_tensor_tensor(
                        sd, ps2[:, :cs], 1.0 / DIM, msq,
                        op0=ALU.mult, op1=ALU.subtract)
                    nc.scalar.activation(sd, sd, AF.Sqrt, bias=eps_sb, scale=1.0)
                    rsig = stp.tile([P, QC], F32, tag="rsig", name=f"rsig{lbl}{lo}")[:, :cs]
                    nc.vector.reciprocal_approx_fast(rsig, sd)
                    mub = stp.tile([P, QC], BF16, tag="mub", name=f"mub{lbl}{lo}")[:, :cs]
                    nc.vector.tensor_copy(mub, mu)
                    rsb = stp.tile([P, QC], BF16, tag="rsb", name=f"rsb{lbl}{lo}")[:, :cs]
                    nc.vector.tensor_copy(rsb, rsig)
                    # t = (x - mu) * rsig over the whole [128, 8, cs] block
                    t = sqp.tile([P, KO, QC], BF16, tag="t", name=f"t{lbl}{lo}")
                    mu_b = mub.unsqueeze(1).broadcast_to([P, KO, cs])
                    rs_b = rsb.unsqueeze(1).broadcast_to([P, KO, cs])
                    nc.vector.tensor_sub(t[:, :, :cs], in_sb[:, :, qs], mu_b)
                    nc.vector.tensor_mul(t[:, :, :cs], t[:, :, :cs], rs_b)
                    for do in range(KO):
                        # out = t * g + b  (per-partition affine on ACT)
                        emit_out(do, qs, t[:, do, :cs],
                                 g_sb[:, do:do + 1], b_sb[:, do:do + 1])

                o1t = actp.tile([P, KO, NX], BF16, tag="big", name="o1t")

                def emit_o1(do, qs, t, g_col, b_col):
                    nc.scalar.activation(o1t[:, do, qs], t, AF.Identity,
                                         bias=b_col, scale=g_col)

                z2t = actp.tile([P, KO, NX], BF16, tag="big", name="z2t")

                def oproj_qc(qc):
                    # HT[n, q] = sum_d Wo[d, n] O1T[d, q]; z2 = o1+relu(H+bo)
                    # (relu into a staging tile so the residual add is one
                    # merged DVE op per 512-token half)
                    qs = slice(qc * QC, (qc + 1) * QC)
                    ht = outp.tile([P, KO, QC], BF16, tag="ht", name=f"ht{qc}")
                    for no in range(KO):
                        po = pp3.tile([P, QC], F32, tag="po", name=f"po{no}{qc}")
                        for k in range(KO):
                            nc.tensor.matmul(
                                po,
                                lhsT=wo_sb[:, k, no * P:(no + 1) * P],
                                rhs=o1t[:, k, qs],
                                start=(k == 0), stop=(k == KO - 1))
                        nc.scalar.activation(ht[:, no, :], po, AF.Relu,
                                             bias=bo_sb[:, no:no + 1], scale=1.0)
                    nc.vector.tensor_add(z2t[:, :, qs], ht, o1t[:, :, qs])

                def emit_o2(do, qs, t, g_col, b_col):
                    o = outp.tile([P, QC], BF16, tag="o", name=f"o{do}")[:, :t.shape[-1]]
                    nc.scalar.activation(o, t, AF.Identity,
                                         bias=b_col, scale=g_col)
                    nc.sync.dma_start(ot3[:, do, qs], o)

                for qc in range(NQC):
                    layernorm_chunk(zt, qc * QC, (qc + 1) * QC,
                                    g1_sb, b1_sb, emit_o1, "a")
                if DEBUG:
                    nc.sync.dma_start(dbg["d_o1t"], o1t)
                for qc in range(NQC):
                    oproj_qc(qc)
                if DEBUG:
                    nc.sync.dma_start(dbg["d_z2t"], z2t)
                # LN2 in shrinking chunks: the early big chunk overlaps the
                # second O-proj half, the small final chunk keeps the serial
                # drain at the very end of the kernel short
                for lo, hi in ((0, 512), (512, 768), (768, 1024)):
                    layernorm_chunk(z2t, lo, hi, g2_sb, b2_sb, emit_o2, "b")

    nc.compile()
    return nc


_CACHE = {}


def _get_nc():
    if "nc" not in _CACHE:
        _CACHE["nc"] = _build()
    return _CACHE["nc"]


def make_in_maps(X, Y, mask, Wq, bq, Wk, bk, Wv, bv, Wo, bo, g1, b1, g2, b2):
    bf = lambda a: np.ascontiguousarray(
        np.asarray(a, dtype=np.float32).astype(ml_dtypes.bfloat16))
    f = lambda a: np.ascontiguousarray(np.asarray(a, dtype=np.float32))
    shared = {
        "Wq": bf(Wq), "Wk": bf(Wk), "Wv": bf(Wv), "Wo": bf(Wo),
        "bq": f(bq), "bk": f(bk), "bv": f(bv), "bo": f(bo),
        "g1": f(g1), "b1": f(b1), "g2": f(g2), "b2": f(b2),
    }
    X = np.asarray(X, dtype=np.float32)
    Y = np.asarray(Y, dtype=np.float32)
    mask = np.asarray(mask)
    in_maps = []
    for b in range(8):
        mb = np.where(mask[b], np.float32(-1e4), np.float32(0.0)).astype(np.float32)
        in_maps.append({
            "XT": bf(X[b].T),
            "YT": bf(Y[b].T),
            "MB": mb,
            **shared,
        })
    return in_maps


def kernel(X, Y, mask, Wq, bq, Wk, bk, Wv, bv, Wo, bo, g1, b1, g2, b2,
           _trace=False):
    nc = _get_nc()
    in_maps = make_in_maps(X, Y, mask, Wq, bq, Wk, bk, Wv, bv, Wo, bo,
                           g1, b1, g2, b2)
    res = run_bass_kernel_spmd(nc, in_maps, core_ids=list(range(8)),
                               trace=_trace)
    out = np.stack([np.asarray(res.results[b]["OT"]).astype(np.float32).T
                    for b in range(8)])
    out = np.ascontiguousarray(out)
    if _trace:
        return out, res
    return out


# revision 23
# speedup vs baseline: 1.5619x; 1.0348x over previous
"""Trainium2 Bass kernel for a masked-attention block (MAB).

Computation (per batch element):
    Q = X@Wq + bq ; K = Y@Wk + bk ; V = Y@Wv + bv
    logits = per-head Qh@Kh^T / 32, masked keys -> -inf, softmax over keys
    attn   = A @ Vh (concat heads)
    O1 = LN(Q + attn; g1,b1)
    O  = LN(O1 + relu(O1@Wo + bo); g2,b2)

Sharding: pure data-parallel, one batch element per NeuronCore (B=8 = 8 cores).

On-device dataflow is "feature-major": activations live in SBUF transposed
([model_dim -> 8x128 partitions, token -> free]).  With weights in natural
layout every matmul chains without any transposes.  All matmul operands are
bf16 (PE rate is identical to fp32r, but: half the DMA bytes, FWL-accelerated
LDWEIGHTS, and 2x packed DVE ops); PSUM accumulation stays fp32.

Schedule (engines run their queues in program order; this ordering is the
software pipeline):
  pre-loop : V proj (natural layout), Q proj        [PE; ACT does epilogues]
  loop h   : denom tree for h-1 [DVE], K proj h [PE, DVE epi], logits h [PE],
             exp h [ACT], AV h-1 [PE], attn epilogue h-1 [DVE]
  tail     : LN1 -> O proj -> LN2 per 512-token half, pipelined across
             PE (stats matmuls, proj) / DVE (elementwise) / ACT (affine)

The softmax denominator is a partition-dim reduction done as a bf16 pairwise
tree on DVE (frees the PE of ~65k ones-matmul columns); LayerNorm stats stay
as all-ones stationary matmuls (cheap, and they broadcast for free).

The host transposes X/Y on the way in and the output on the way out, converts
everything the matmuls touch to bf16, and turns the bool mask into an
additive f32 bias (0 / -1e4) consumed by the exp activation.
"""

import math
import numpy as np
from contextlib import ExitStack

import ml_dtypes

import concourse.bass as bass
import concourse.mybir as mybir
import concourse.tile as tile
from concourse import bacc
from concourse.bass_utils import run_bass_kernel_spmd

P = 128
NX = 1024
NY = 1024
DIM = 1024
H = 8
KO = DIM // P          # 8 partition sub-tiles of the model dim
QC = 512               # moving-operand chunk
NQC = NX // QC         # 2
F32 = mybir.dt.float32
BF16 = mybir.dt.bfloat16
AF = mybir.ActivationFunctionType
ALU = mybir.AluOpType
SCALE = 1.0 / 32.0     # 1/sqrt(DIM)
EPS = 1e-5
DEBUG = False          # adds intermediate-tensor DRAM dumps (debugging only)


def _build():
    nc = bacc.Bacc("TRN2", target_bir_lowering=False, debug=False,
                   enable_asserts=False)

    # ---- DRAM I/O (per-core shapes) ----
    XT = nc.dram_tensor("XT", [DIM, NX], BF16, kind="ExternalInput").ap()
    YT = nc.dram_tensor("YT", [DIM, NY], BF16, kind="ExternalInput").ap()
    MB = nc.dram_tensor("MB", [NY], F32, kind="ExternalInput").ap()
    Wd = {}
    for w in ("Wq", "Wk", "Wv", "Wo"):
        Wd[w] = nc.dram_tensor(w, [DIM, DIM], BF16, kind="ExternalInput").ap()
    Vecs = {}
    for vname in ("bq", "bk", "bv", "bo", "g1", "b1", "g2", "b2"):
        Vecs[vname] = nc.dram_tensor(vname, [DIM], F32, kind="ExternalInput").ap()
    OT = nc.dram_tensor("OT", [DIM, NX], BF16, kind="ExternalOutput").ap()

    xt3 = XT.rearrange("(ko p) q -> p ko q", p=P)
    yt3 = YT.rearrange("(ko p) q -> p ko q", p=P)
    wq3 = Wd["Wq"].rearrange("(ko p) d -> p ko d", p=P)
    wk3 = Wd["Wk"].rearrange("(ko p) d -> p ko d", p=P)
    wv3 = Wd["Wv"].rearrange("(ko p) d -> p ko d", p=P)
    wo3 = Wd["Wo"].rearrange("(ko p) d -> p ko d", p=P)
    ot3 = OT.rearrange("(do p) q -> p do q", p=P)

    dbg = {}
    if DEBUG:
        for nm, shp, dt in [("d_qt", [P, KO, NX], BF16),
                            ("d_ktm0", [P, NY], BF16),
                            ("d_vm", [P, KO, DIM], BF16),
                            ("d_et00", [P, NX], BF16),
                            ("d_rc0", [P, NX], F32),
                            ("d_zt", [P, KO, NX], BF16),
                            ("d_o1t", [P, KO, NX], BF16),
                            ("d_z2t", [P, KO, NX], BF16)]:
            dbg[nm] = nc.dram_tensor(nm, shp, dt, kind="ExternalOutput").ap()

    with tile.TileContext(nc) as tc:
        with ExitStack() as octx:
            const = octx.enter_context(tc.tile_pool(name="const", bufs=1))
            persist = octx.enter_context(tc.tile_pool(name="persist", bufs=1))
            actp = octx.enter_context(tc.tile_pool(name="act", bufs=3))

            # ---- constants (issue the small DMAs first on the sync queue) ----
            ones_bf = const.tile([P, P], BF16, tag="onesbf", name="ones_bf")
            nc.vector.memset(ones_bf, 1.0)
            eps_sb = const.tile([P, 1], F32, tag="eps", name="eps_sb")
            nc.vector.memset(eps_sb, EPS)

            def vec_pko(name):
                t = const.tile([P, KO], F32, tag=f"v_{name}", name=f"{name}_sb")
                nc.sync.dma_start(t, Vecs[name].rearrange("(ko p) -> p ko", p=P))
                return t

            mb_sb = const.tile([P, KO], F32, tag="v_mb", name="mb_sb")
            nc.sync.dma_start(mb_sb, MB.rearrange("(ko p) -> p ko", p=P))
            bq_sb = vec_pko("bq")
            bk_sb = vec_pko("bk")
            bv_sb = vec_pko("bv")
            bo_sb = vec_pko("bo")
            g1_sb = vec_pko("g1")
            b1_sb = vec_pko("b1")
            g2_sb = vec_pko("g2")
            b2_sb = vec_pko("b2")

            # ---- persistent tensors ----
            yt = persist.tile([P, KO, NY], BF16, tag="yt", name="yt")
            vm = persist.tile([P, KO, DIM], BF16, tag="vm", name="vm")
            wo_sb = persist.tile([P, KO, DIM], BF16, tag="wo", name="wo_sb")
            wkp = octx.enter_context(tc.tile_pool(name="wkp", bufs=2))
            ktmp = octx.enter_context(tc.tile_pool(name="ktmp", bufs=2))

            # big feature-major activation tiles (rotate through 3 slots)
            qt = actp.tile([P, KO, NX], BF16, tag="big", name="qt")

            # ================= Phase 1: V and Q projections =================
            with tc.tile_pool(name="io", bufs=1) as iop, \
                 tc.tile_pool(name="gp1", bufs=8, space="PSUM") as pp:
                wv_sb = iop.tile([P, KO, DIM], BF16, tag="wv", name="wv_sb")
                xt = iop.tile([P, KO, NX], BF16, tag="xt", name="xt")
                wq_sb = iop.tile([P, KO, DIM], BF16, tag="wq", name="wq_sb")
                # per-k chunked DMAs, interleaved by priority so the V proj
                # can start as soon as the first (yt, wv) chunk pair lands
                for k in range(KO):
                    nc.sync.dma_start(yt[:, k, :], yt3[:, k, :])
                    nc.sync.dma_start(wv_sb[:, k, :], wv3[:, k, :])
                for k in range(KO):
                    nc.sync.dma_start(xt[:, k, :], xt3[:, k, :])
                    nc.sync.dma_start(wq_sb[:, k, :], wq3[:, k, :])
                wk_tiles = []
                for h in range(2):
                    wkt = wkp.tile([P, KO, P], BF16, tag="wk", name=f"wk{h}")
                    nc.sync.dma_start(wkt, wk3[:, :, h * P:(h + 1) * P])
                    wk_tiles.append(wkt)
                nc.sync.dma_start(wo_sb, wo3)

                # V in natural (token-major) layout: V[y, n] = sum_k Y[y,k] Wv[k,n]
                # (bias bv is NOT added here: softmax rows sum to 1, so it is
                # folded into the attention epilogue instead)
                for yo in range(KO):
                    pss = [pp.tile([P, QC], F32, tag="ps", name=f"ps_v{yo}{ng}")
                           for ng in range(2)]
                    for k in range(KO):
                        for ng in range(2):
                            ns = slice(ng * QC, (ng + 1) * QC)
                            nc.tensor.matmul(
                                pss[ng],
                                lhsT=yt[:, k, yo * P:(yo + 1) * P],
                                rhs=wv_sb[:, k, ns],
                                start=(k == 0), stop=(k == KO - 1))
                    for ng in range(2):
                        ns = slice(ng * QC, (ng + 1) * QC)
                        nc.scalar.copy(vm[:, yo, ns], pss[ng])

                # Q feature-major: qt[p, do, q] = sum_k Wq[k, d] xt[k, q] + bq
                for do in range(KO):
                    for qc in range(NQC):
                        qs = slice(qc * QC, (qc + 1) * QC)
                        ps = pp.tile([P, QC], F32, tag="ps", name=f"ps_q{do}{qc}")
                        for k in range(KO):
                            nc.tensor.matmul(
                                ps,
                                lhsT=wq_sb[:, k, do * P:(do + 1) * P],
                                rhs=xt[:, k, qs],
                                start=(k == 0), stop=(k == KO - 1))
                        nc.scalar.activation(
                            qt[:, do, qs], ps, AF.Identity,
                            bias=bq_sb[:, do:do + 1], scale=1.0)

            if DEBUG:
                nc.sync.dma_start(dbg["d_qt"], qt)
                nc.sync.dma_start(dbg["d_vm"], vm)

            # ================= Phase 2: K proj + attention (pipelined) ======
            zt = actp.tile([P, KO, NX], BF16, tag="big", name="zt")

            with tc.tile_pool(name="kq", bufs=2, space="PSUM") as kqp, \
                 tc.tile_pool(name="lgp", bufs=2, space="PSUM") as lgp, \
                 tc.tile_pool(name="avp", bufs=1, space="PSUM") as avp, \
                 tc.tile_pool(name="exp", bufs=17) as ep, \
                 tc.tile_pool(name="prs", bufs=7) as prp, \
                 tc.tile_pool(name="den", bufs=2) as dnp, \
                 tc.tile_pool(name="rcp", bufs=2) as rcp:

                def kproj(h):
                    # K slab h: ktm_h[p, y] = sum_k Wk[k, h*128+p] yt[k, y] + bk
                    ktm_h = ktmp.tile([P, NY], BF16, tag="ktm", name=f"ktm{h}")
                    for qc in range(NQC):
                        qs = slice(qc * QC, (qc + 1) * QC)
                        pk = kqp.tile([P, QC], F32, tag="pk", name=f"pk{h}{qc}")
                        for k in range(KO):
                            nc.tensor.matmul(
                                pk, lhsT=wk_tiles[h][:, k, :], rhs=yt[:, k, qs],
                                start=(k == 0), stop=(k == KO - 1))
                        nc.vector.tensor_scalar(
                            ktm_h[:, qs], pk, bk_sb[:, h:h + 1], None,
                            op0=ALU.add)
                    if h + 2 < H:
                        # prefetch the h+2 weight chunk; issued after this
                        # head's matmuls so the 2-deep pool rotation can't
                        # clobber a chunk that still has unissued readers
                        wkt = wkp.tile([P, KO, P], BF16, tag="wk", name=f"wk{h+2}")
                        nc.sync.dma_start(wkt, wk3[:, :, (h + 2) * P:(h + 3) * P])
                        wk_tiles.append(wkt)
                    return ktm_h

                def logits_exp(h, ktm_h):
                    # logitsT[k, q] = sum_d KT_h[d, k] QT_h[d, q]; exp with
                    # mask bias per key (partition) and 1/32 scale
                    et = [ep.tile([P, NX], BF16, tag="exp", name=f"et{h}_{k}")
                          for k in range(KO)]
                    for kt in range(KO):
                        pl = lgp.tile([P, NX], F32, tag="lg", name=f"pl{h}{kt}")
                        for qc in range(NQC):
                            qs = slice(qc * QC, (qc + 1) * QC)
                            nc.tensor.matmul(
                                pl[:, qs],
                                lhsT=ktm_h[:, kt * P:(kt + 1) * P],
                                rhs=qt[:, h, qs],
                                start=True, stop=True)
                        nc.scalar.activation(
                            et[kt], pl, AF.Exp,
                            bias=mb_sb[:, kt:kt + 1], scale=SCALE)
                    return et

                def denom_tree(h, et):
                    # softmax denominator, part 1 (DVE only): cross-tile
                    # pairwise tree (bf16 2x) collapses the 8 key-slab tiles
                    # into one; issued at iteration start to fill the DVE
                    # bubble while the PE runs this head's K projection
                    l1 = [prp.tile([P, NX], BF16, tag="pr", name=f"l1_{h}{i}")
                          for i in range(4)]
                    for i in range(4):
                        nc.vector.tensor_add(l1[i], et[2 * i], et[2 * i + 1])
                    l2 = [prp.tile([P, NX], BF16, tag="pr", name=f"l2_{h}{i}")
                          for i in range(2)]
                    for i in range(2):
                        nc.vector.tensor_add(l2[i], l1[2 * i], l1[2 * i + 1])
                    den = dnp.tile([P, NX], BF16, tag="dn", name=f"den{h}")
                    nc.vector.tensor_add(den, l2[0], l2[1])
                    return den

                def denom_finish(h, den):
                    # part 2: a small all-ones matmul does the in-slab
                    # 128-partition reduction (and broadcasts for free)
                    rc = rcp.tile([P, NX], F32, tag="rc", name=f"rc{h}")
                    for qc in range(NQC):
                        qs = slice(qc * QC, (qc + 1) * QC)
                        pr = kqp.tile([P, QC], F32, tag="pk", name=f"pr{h}{qc}")
                        nc.tensor.matmul(pr, lhsT=ones_bf, rhs=den[:, qs],
                                         start=True, stop=True)
                        nc.vector.reciprocal_approx_fast(rc[:, qs], pr)
                    return rc

                def av_epi(h, et, rc):
                    # attnT_h[d, q] = sum_k V[k, d_h] expT[k, q]; normalize by
                    # the softmax denominator, add bv (softmax rows sum to 1,
                    # so + bv after normalize == bias inside the V proj) and
                    # the Q residual
                    pa = avp.tile([P, NX], F32, tag="av", name=f"pa{h}")
                    for kt in range(KO):
                        for qc in range(NQC):
                            qs = slice(qc * QC, (qc + 1) * QC)
                            nc.tensor.matmul(
                                pa[:, qs],
                                lhsT=vm[:, kt, h * P:(h + 1) * P],
                                rhs=et[kt][:, qs],
                                start=(kt == 0), stop=(kt == KO - 1))
                    nc.vector.tensor_mul(zt[:, h, :], pa, rc)
                    nc.vector.scalar_tensor_tensor(
                        zt[:, h, :], zt[:, h, :], bv_sb[:, h:h + 1],
                        qt[:, h, :], op0=ALU.add, op1=ALU.add)

                # software pipeline: head h's K proj / logits / exp overlap
                # head h-1's denominator (DVE tree + tiny ones-matmul) and
                # AV (PE).  Issue order is chosen per engine queue: the DVE
                # tree first (ready at iteration start), the PE denominator
                # matmuls after the logits matmuls (so the in-order PE queue
                # never waits on the tree).
                prev = None
                den_prev = None
                for h in range(H):
                    if prev is not None:
                        den_prev = denom_tree(h - 1, prev)
                    ktm_h = kproj(h)
                    et = logits_exp(h, ktm_h)
                    if DEBUG and h == 0:
                        nc.sync.dma_start(dbg["d_ktm0"], ktm_h)
                        nc.sync.dma_start(dbg["d_et00"], et[0])
                    if prev is not None:
                        rc_prev = denom_finish(h - 1, den_prev)
                        if DEBUG and h == 1:
                            nc.sync.dma_start(dbg["d_rc0"], rc_prev)
                        av_epi(h - 1, prev, rc_prev)
                    prev = et
                den_prev = denom_tree(H - 1, prev)
                rc_prev = denom_finish(H - 1, den_prev)
                av_epi(H - 1, prev, rc_prev)
                if DEBUG:
                    nc.sync.dma_start(dbg["d_zt"], zt)

            # ================= Phase 3: LN1 -> O proj + LN2 =================
            # LayerNorm over the model dim (partition direction): stats via
            # all-ones stationary matmuls (free broadcast), elementwise on
            # DVE, gamma/beta affine on ACT.
            with tc.tile_pool(name="sqp", bufs=2) as sqp, \
                 tc.tile_pool(name="stp", bufs=2) as stp, \
                 tc.tile_pool(name="out", bufs=4) as outp, \
                 tc.tile_pool(name="spp", bufs=2, space="PSUM") as spp, \
                 tc.tile_pool(name="gp3", bufs=4, space="PSUM") as pp3:

                def layernorm_chunk(in_sb, lo, hi, g_sb, b_sb, emit_out, lbl):
                    cs = hi - lo
                    qs = slice(lo, hi)
                    pmu = spp.tile([P, QC], F32, tag="pmu", name=f"pmu{lbl}{lo}")
                    for do in range(KO):
                        nc.tensor.matmul(pmu[:, :cs], lhsT=ones_bf,
                                         rhs=in_sb[:, do, qs],
                                         start=(do == 0), stop=(do == KO - 1))
                    sq = sqp.tile([P, KO, QC], BF16, tag="sq", name=f"sq{lbl}{lo}")
                    nc.scalar.square(sq[:, :, :cs], in_sb[:, :, qs])
                    ps2 = spp.tile([P, QC], F32, tag="ps2", name=f"ps2{lbl}{lo}")
                    for do in range(KO):
                        nc.tensor.matmul(ps2[:, :cs], lhsT=ones_bf,
                                         rhs=sq[:, do, :cs],
                                         start=(do == 0), stop=(do == KO - 1))
                    mu = stp.tile([P, QC], F32, tag="mu", name=f"mu{lbl}{lo}")[:, :cs]
                    nc.vector.tensor_scalar_mul(mu, pmu[:, :cs], 1.0 / DIM)
                    msq = stp.tile([P, QC], F32, tag="msq", name=f"msq{lbl}{lo}")[:, :cs]
                    nc.vector.tensor_mul(msq, mu, mu)
                    sd = stp.tile([P, QC], F32, tag="sd", name=f"sd{lbl}{lo}")[:, :cs]
                    nc.vector.scalar_tensor_tensor(
                        sd, ps2[:, :cs], 1.0 / DIM, msq,
                        op0=ALU.mult, op1=ALU.subtract)
                    nc.scalar.activation(sd, sd, AF.Sqrt, bias=eps_sb, scale=1.0)
                    rsig = stp.tile([P, QC], F32, tag="rsig", name=f"rsig{lbl}{lo}")[:, :cs]
                    nc.vector.reciprocal_approx_fast(rsig, sd)
                    mub = stp.tile([P, QC], BF16, tag="mub", name=f"mub{lbl}{lo}")[:, :cs]
                    nc.vector.tensor_copy(mub, mu)
                    rsb = stp.tile([P, QC], BF16, tag="rsb", name=f"rsb{lbl}{lo}")[:, :cs]
                    nc.vector.tensor_copy(rsb, rsig)
                    # t = (x - mu) * rsig over the whole [128, 8, cs] block
                    t = sqp.tile([P, KO, QC], BF16, tag="t", name=f"t{lbl}{lo}")
                    mu_b = mub.unsqueeze(1).broadcast_to([P, KO, cs])
                    rs_b = rsb.unsqueeze(1).broadcast_to([P, KO, cs])
                    nc.vector.tensor_sub(t[:, :, :cs], in_sb[:, :, qs], mu_b)
                    nc.vector.tensor_mul(t[:, :, :cs], t[:, :, :cs], rs_b)
                    for do in range(KO):
                        # out = t * g + b  (per-partition affine, DVE TS 4x)
                        emit_out(do, qs, t[:, do, :cs],
                                 g_sb[:, do:do + 1], b_sb[:, do:do + 1])

                o1t = actp.tile([P, KO, NX], BF16, tag="big", name="o1t")

                def emit_o1(do, qs, t, g_col, b_col):
                    nc.vector.tensor_scalar(o1t[:, do, qs], t, g_col, b_col,
                                            op0=ALU.mult, op1=ALU.add)

                z2t = actp.tile([P, KO, NX], BF16, tag="big", name="z2t")

                def oproj_qc(qc):
                    # HT[n, q] = sum_d Wo[d, n] O1T[d, q]; z2 = o1+relu(H+bo)
                    # (relu into a staging tile so the residual add is one
                    # merged DVE op per 512-token half)
                    qs = slice(qc * QC, (qc + 1) * QC)
                    ht = outp.tile([P, KO, QC], BF16, tag="ht", name=f"ht{qc}")
                    for no in range(KO):
                        po = pp3.tile([P, QC], F32, tag="po", name=f"po{no}{qc}")
                        for k in range(KO):
                            nc.tensor.matmul(
                                po,
                                lhsT=wo_sb[:, k, no * P:(no + 1) * P],
                                rhs=o1t[:, k, qs],
                                start=(k == 0), stop=(k == KO - 1))
                        nc.scalar.activation(ht[:, no, :], po, AF.Relu,
                                             bias=bo_sb[:, no:no + 1], scale=1.0)
                    nc.vector.tensor_add(z2t[:, :, qs], ht, o1t[:, :, qs])

                def emit_o2(do, qs, t, g_col, b_col):
                    o = outp.tile([P, QC], BF16, tag="o", name=f"o{do}")[:, :t.shape[-1]]
                    nc.vector.tensor_scalar(o, t, g_col, b_col,
                                            op0=ALU.mult, op1=ALU.add)
                    nc.sync.dma_start(ot3[:, do, qs], o)

                for qc in range(NQC):
                    layernorm_chunk(zt, qc * QC, (qc + 1) * QC,
                                    g1_sb, b1_sb, emit_o1, "a")
                if DEBUG:
                    nc.sync.dma_start(dbg["d_o1t"], o1t)
                # interleave the second O-proj half behind LN2's first chunk
                # so LN2's stats matmuls don't queue behind all of O-proj on
                # the in-order PE queue; LN2's shrinking chunks keep the
                # serial drain at the very end short
                oproj_qc(0)
                layernorm_chunk(z2t, 0, 512, g2_sb, b2_sb, emit_o2, "b")
                oproj_qc(1)
                if DEBUG:
                    nc.sync.dma_start(dbg["d_z2t"], z2t)
                for lo, hi in ((512, 768), (768, 1024)):
                    layernorm_chunk(z2t, lo, hi, g2_sb, b2_sb, emit_o2, "b")

    nc.compile()
    return nc


_CACHE = {}


def _get_nc():
    if "nc" not in _CACHE:
        _CACHE["nc"] = _build()
    return _CACHE["nc"]


def make_in_maps(X, Y, mask, Wq, bq, Wk, bk, Wv, bv, Wo, bo, g1, b1, g2, b2):
    bf = lambda a: np.ascontiguousarray(
        np.asarray(a, dtype=np.float32).astype(ml_dtypes.bfloat16))
    f = lambda a: np.ascontiguousarray(np.asarray(a, dtype=np.float32))
    shared = {
        "Wq": bf(Wq), "Wk": bf(Wk), "Wv": bf(Wv), "Wo": bf(Wo),
        "bq": f(bq), "bk": f(bk), "bv": f(bv), "bo": f(bo),
        "g1": f(g1), "b1": f(b1), "g2": f(g2), "b2": f(b2),
    }
    X = np.asarray(X, dtype=np.float32)
    Y = np.asarray(Y, dtype=np.float32)
    mask = np.asarray(mask)
    in_maps = []
    for b in range(8):
        mb = np.where(mask[b], np.float32(-1e4), np.float32(0.0)).astype(np.float32)
        in_maps.append({
            "XT": bf(X[b].T),
            "YT": bf(Y[b].T),
            "MB": mb,
            **shared,
        })
    return in_maps


def kernel(X, Y, mask, Wq, bq, Wk, bk, Wv, bv, Wo, bo, g1, b1, g2, b2,
           _trace=False):
    nc = _get_nc()
    in_maps = make_in_maps(X, Y, mask, Wq, bq, Wk, bk, Wv, bv, Wo, bo,
                           g1, b1, g2, b2)
    res = run_bass_kernel_spmd(nc, in_maps, core_ids=list(range(8)),
                               trace=_trace)
    out = np.stack([np.asarray(res.results[b]["OT"]).astype(np.float32).T
                    for b in range(8)])
    out = np.ascontiguousarray(out)
    if _trace:
        return out, res
    return out


# revision 30
# speedup vs baseline: 1.6761x; 1.0731x over previous
"""Trainium2 Bass kernel for a masked-attention block (MAB).

Computation (per batch element):
    Q = X@Wq + bq ; K = Y@Wk + bk ; V = Y@Wv + bv
    logits = per-head Qh@Kh^T / 32, masked keys -> -inf, softmax over keys
    attn   = A @ Vh (concat heads)
    O1 = LN(Q + attn; g1,b1)
    O  = LN(O1 + relu(O1@Wo + bo); g2,b2)

Sharding: pure data-parallel, one batch element per NeuronCore (B=8 = 8 cores).

On-device dataflow is "feature-major": activations live in SBUF transposed
([model_dim -> 8x128 partitions, token -> free]).  With weights in natural
layout every matmul chains without any transposes.  All matmul operands are
bf16 (PE rate is identical to fp32r, but: half the DMA bytes, FWL-accelerated
LDWEIGHTS, and 2x packed DVE ops); PSUM accumulation stays fp32.

Schedule (engines run their queues in program order; this ordering is the
software pipeline):
  pre-loop : V proj (natural layout), Q proj        [PE; ACT does epilogues]
  loop h   : denom tree for h-1 [DVE], K proj h [PE, DVE epi], logits h [PE],
             exp h [ACT], AV h-1 [PE], attn epilogue h-1 [DVE]
  tail     : LN1 -> O proj -> LN2 per 512-token half, pipelined across
             PE (stats matmuls, proj) / DVE (elementwise) / ACT (affine)

The softmax denominator is a partition-dim reduction done as a bf16 pairwise
tree on DVE (frees the PE of ~65k ones-matmul columns); LayerNorm stats stay
as all-ones stationary matmuls (cheap, and they broadcast for free).

The host transposes X/Y on the way in and the output on the way out, converts
everything the matmuls touch to bf16, and turns the bool mask into an
additive f32 bias (0 / -1e4) consumed by the exp activation.
"""

import math
import numpy as np
from contextlib import ExitStack

import ml_dtypes

import concourse.bass as bass
import concourse.mybir as mybir
import concourse.tile as tile
from concourse import bacc
from concourse.bass_utils import run_bass_kernel_spmd

P = 128
NX = 1024
NY = 1024
DIM = 1024
H = 8
KO = DIM // P          # 8 partition sub-tiles of the model dim
QC = 512               # moving-operand chunk
NQC = NX // QC         # 2
F32 = mybir.dt.float32
BF16 = mybir.dt.bfloat16
AF = mybir.ActivationFunctionType
ALU = mybir.AluOpType
SCALE = 1.0 / 32.0     # 1/sqrt(DIM)
EPS = 1e-5
DEBUG = False          # adds intermediate-tensor DRAM dumps (debugging only)


def _build():
    nc = bacc.Bacc("TRN2", target_bir_lowering=False, debug=False,
                   enable_asserts=False)

    # ---- DRAM I/O (per-core shapes) ----
    XT = nc.dram_tensor("XT", [DIM, NX], BF16, kind="ExternalInput").ap()
    YT = nc.dram_tensor("YT", [DIM, NY], BF16, kind="ExternalInput").ap()
    MB = nc.dram_tensor("MB", [NY], F32, kind="ExternalInput").ap()
    Wd = {}
    for w in ("Wq", "Wk", "Wv", "Wo"):
        Wd[w] = nc.dram_tensor(w, [DIM, DIM], BF16, kind="ExternalInput").ap()
    Vecs = {}
    for vname in ("bq", "bk", "bv", "bo", "g1", "b1", "g2", "b2"):
        Vecs[vname] = nc.dram_tensor(vname, [DIM], F32, kind="ExternalInput").ap()
    OT = nc.dram_tensor("OT", [DIM, NX], BF16, kind="ExternalOutput").ap()

    xt3 = XT.rearrange("(ko p) q -> p ko q", p=P)
    yt3 = YT.rearrange("(ko p) q -> p ko q", p=P)
    wq3 = Wd["Wq"].rearrange("(ko p) d -> p ko d", p=P)
    wk3 = Wd["Wk"].rearrange("(ko p) d -> p ko d", p=P)
    wv3 = Wd["Wv"].rearrange("(ko p) d -> p ko d", p=P)
    wo3 = Wd["Wo"].rearrange("(ko p) d -> p ko d", p=P)
    ot3 = OT.rearrange("(do p) q -> p do q", p=P)

    dbg = {}
    if DEBUG:
        for nm, shp, dt in [("d_qt", [P, KO, NX], BF16),
                            ("d_ktm0", [P, NY], BF16),
                            ("d_vm", [P, KO, DIM], BF16),
                            ("d_et00", [P, NX], BF16),
                            ("d_rc0", [P, NX], F32),
                            ("d_zt", [P, KO, NX], BF16),
                            ("d_o1t", [P, KO, NX], BF16),
                            ("d_z2t", [P, KO, NX], BF16)]:
            dbg[nm] = nc.dram_tensor(nm, shp, dt, kind="ExternalOutput").ap()

    with tile.TileContext(nc) as tc:
        with ExitStack() as octx:
            const = octx.enter_context(tc.tile_pool(name="const", bufs=1))
            persist = octx.enter_context(tc.tile_pool(name="persist", bufs=1))
            actp = octx.enter_context(tc.tile_pool(name="act", bufs=3))

            # ---- constants (issue the small DMAs first on the sync queue) ----
            ones_bf = const.tile([P, P], BF16, tag="onesbf", name="ones_bf")
            nc.vector.memset(ones_bf, 1.0)
            eps_sb = const.tile([P, 1], F32, tag="eps", name="eps_sb")
            nc.vector.memset(eps_sb, EPS)

            # vector constants: tiles now, DMAs issued after the big input
            # tensors (nothing reads them before ~45us)
            def vec_tile(name):
                return const.tile([P, KO], F32, tag=f"v_{name}", name=f"{name}_sb")

            vec_names = ("bq", "bk", "bv", "bo", "g1", "b1", "g2", "b2")
            vec_sb = {n: vec_tile(n) for n in vec_names}
            mb_sb = const.tile([P, KO], F32, tag="v_mb", name="mb_sb")
            bq_sb, bk_sb, bv_sb, bo_sb = (vec_sb[n] for n in vec_names[:4])
            g1_sb, b1_sb, g2_sb, b2_sb = (vec_sb[n] for n in vec_names[4:])

            def issue_vec_dmas():
                nc.sync.dma_start(mb_sb, MB.rearrange("(ko p) -> p ko", p=P))
                for n in vec_names:
                    nc.sync.dma_start(
                        vec_sb[n], Vecs[n].rearrange("(ko p) -> p ko", p=P))

            # ---- persistent tensors ----
            yt = persist.tile([P, KO, NY], BF16, tag="yt", name="yt")
            vm = persist.tile([P, KO, DIM], BF16, tag="vm", name="vm")
            wo_sb = persist.tile([P, KO, DIM], BF16, tag="wo", name="wo_sb")
            wkp = octx.enter_context(tc.tile_pool(name="wkp", bufs=2))
            ktmp = octx.enter_context(tc.tile_pool(name="ktmp", bufs=2))

            # big feature-major activation tiles (rotate through 3 slots)
            qt = actp.tile([P, KO, NX], BF16, tag="big", name="qt")

            # ================= Phase 1: V and Q projections =================
            with tc.tile_pool(name="io", bufs=1) as iop, \
                 tc.tile_pool(name="gp1", bufs=8, space="PSUM") as pp:
                wv_sb = iop.tile([P, KO, DIM], BF16, tag="wv", name="wv_sb")
                xt = iop.tile([P, KO, NX], BF16, tag="xt", name="xt")
                wq_sb = iop.tile([P, KO, DIM], BF16, tag="wq", name="wq_sb")
                # per-k chunked DMAs, interleaved by priority so the V proj
                # can start as soon as the first (yt, wv) chunk pair lands
                for k in range(KO):
                    nc.sync.dma_start(yt[:, k, :], yt3[:, k, :])
                    nc.sync.dma_start(wv_sb[:, k, :], wv3[:, k, :])
                for k in range(KO):
                    nc.sync.dma_start(xt[:, k, :], xt3[:, k, :])
                    nc.sync.dma_start(wq_sb[:, k, :], wq3[:, k, :])
                wk_tiles = []
                for h in range(2):
                    wkt = wkp.tile([P, KO, P], BF16, tag="wk", name=f"wk{h}")
                    nc.sync.dma_start(wkt, wk3[:, :, h * P:(h + 1) * P])
                    wk_tiles.append(wkt)
                issue_vec_dmas()
                nc.sync.dma_start(wo_sb, wo3)

                # V in natural (token-major) layout: V[y, n] = sum_k Y[y,k] Wv[k,n]
                # (bias bv is NOT added here: softmax rows sum to 1, so it is
                # folded into the attention epilogue instead)
                for yo in range(KO):
                    pss = [pp.tile([P, QC], F32, tag="ps", name=f"ps_v{yo}{ng}")
                           for ng in range(2)]
                    for k in range(KO):
                        for ng in range(2):
                            ns = slice(ng * QC, (ng + 1) * QC)
                            nc.tensor.matmul(
                                pss[ng],
                                lhsT=yt[:, k, yo * P:(yo + 1) * P],
                                rhs=wv_sb[:, k, ns],
                                start=(k == 0), stop=(k == KO - 1))
                    for ng in range(2):
                        ns = slice(ng * QC, (ng + 1) * QC)
                        nc.scalar.copy(vm[:, yo, ns], pss[ng])

                # Q feature-major: qt[p, do, q] = sum_k Wq[k, d] xt[k, q] + bq
                for do in range(KO):
                    for qc in range(NQC):
                        qs = slice(qc * QC, (qc + 1) * QC)
                        ps = pp.tile([P, QC], F32, tag="ps", name=f"ps_q{do}{qc}")
                        for k in range(KO):
                            nc.tensor.matmul(
                                ps,
                                lhsT=wq_sb[:, k, do * P:(do + 1) * P],
                                rhs=xt[:, k, qs],
                                start=(k == 0), stop=(k == KO - 1))
                        nc.scalar.activation(
                            qt[:, do, qs], ps, AF.Identity,
                            bias=bq_sb[:, do:do + 1], scale=1.0)

            if DEBUG:
                nc.sync.dma_start(dbg["d_qt"], qt)
                nc.sync.dma_start(dbg["d_vm"], vm)

            # ================= Phase 2: K proj + attention (pipelined) ======
            zt = actp.tile([P, KO, NX], BF16, tag="big", name="zt")

            with tc.tile_pool(name="kq", bufs=2, space="PSUM") as kqp, \
                 tc.tile_pool(name="lgp", bufs=2, space="PSUM") as lgp, \
                 tc.tile_pool(name="avp", bufs=1, space="PSUM") as avp, \
                 tc.tile_pool(name="exp", bufs=17) as ep, \
                 tc.tile_pool(name="prs", bufs=7) as prp, \
                 tc.tile_pool(name="den", bufs=2) as dnp, \
                 tc.tile_pool(name="rcp", bufs=2) as rcp:

                def kproj(h):
                    # K slab h: ktm_h[p, y] = sum_k Wk[k, h*128+p] yt[k, y] + bk
                    ktm_h = ktmp.tile([P, NY], BF16, tag="ktm", name=f"ktm{h}")
                    for qc in range(NQC):
                        qs = slice(qc * QC, (qc + 1) * QC)
                        pk = kqp.tile([P, QC], F32, tag="pk", name=f"pk{h}{qc}")
                        for k in range(KO):
                            nc.tensor.matmul(
                                pk, lhsT=wk_tiles[h][:, k, :], rhs=yt[:, k, qs],
                                start=(k == 0), stop=(k == KO - 1))
                        nc.vector.tensor_scalar(
                            ktm_h[:, qs], pk, bk_sb[:, h:h + 1], None,
                            op0=ALU.add)
                    if h + 2 < H:
                        # prefetch the h+2 weight chunk; issued after this
                        # head's matmuls so the 2-deep pool rotation can't
                        # clobber a chunk that still has unissued readers
                        wkt = wkp.tile([P, KO, P], BF16, tag="wk", name=f"wk{h+2}")
                        nc.sync.dma_start(wkt, wk3[:, :, (h + 2) * P:(h + 3) * P])
                        wk_tiles.append(wkt)
                    return ktm_h

                def logits_exp(h, ktm_h):
                    # logitsT[k, q] = sum_d KT_h[d, k] QT_h[d, q]; exp with
                    # mask bias per key (partition) and 1/32 scale
                    et = [ep.tile([P, NX], BF16, tag="exp", name=f"et{h}_{k}")
                          for k in range(KO)]
                    for kt in range(KO):
                        pl = lgp.tile([P, NX], F32, tag="lg", name=f"pl{h}{kt}")
                        for qc in range(NQC):
                            qs = slice(qc * QC, (qc + 1) * QC)
                            nc.tensor.matmul(
                                pl[:, qs],
                                lhsT=ktm_h[:, kt * P:(kt + 1) * P],
                                rhs=qt[:, h, qs],
                                start=True, stop=True)
                        nc.scalar.activation(
                            et[kt], pl, AF.Exp,
                            bias=mb_sb[:, kt:kt + 1], scale=SCALE)
                    return et

                def denom_tree(h, et):
                    # softmax denominator, part 1 (DVE only): cross-tile
                    # pairwise tree (bf16 2x) collapses the 8 key-slab tiles
                    # into one; issued at iteration start to fill the DVE
                    # bubble while the PE runs this head's K projection
                    l1 = [prp.tile([P, NX], BF16, tag="pr", name=f"l1_{h}{i}")
                          for i in range(4)]
                    for i in range(4):
                        nc.vector.tensor_add(l1[i], et[2 * i], et[2 * i + 1])
                    l2 = [prp.tile([P, NX], BF16, tag="pr", name=f"l2_{h}{i}")
                          for i in range(2)]
                    for i in range(2):
                        nc.vector.tensor_add(l2[i], l1[2 * i], l1[2 * i + 1])
                    den = dnp.tile([P, NX], BF16, tag="dn", name=f"den{h}")
                    nc.vector.tensor_add(den, l2[0], l2[1])
                    return den

                def denom_finish(h, den):
                    # part 2: a small all-ones matmul does the in-slab
                    # 128-partition reduction (and broadcasts for free)
                    rc = rcp.tile([P, NX], F32, tag="rc", name=f"rc{h}")
                    for qc in range(NQC):
                        qs = slice(qc * QC, (qc + 1) * QC)
                        pr = kqp.tile([P, QC], F32, tag="pk", name=f"pr{h}{qc}")
                        nc.tensor.matmul(pr, lhsT=ones_bf, rhs=den[:, qs],
                                         start=True, stop=True)
                        nc.vector.reciprocal_approx_fast(rc[:, qs], pr)
                    return rc

                def av_epi(h, et, rc):
                    # attnT_h[d, q] = sum_k V[k, d_h] expT[k, q]; normalize by
                    # the softmax denominator, add bv (softmax rows sum to 1,
                    # so + bv after normalize == bias inside the V proj) and
                    # the Q residual
                    pa = avp.tile([P, NX], F32, tag="av", name=f"pa{h}")
                    for kt in range(KO):
                        for qc in range(NQC):
                            qs = slice(qc * QC, (qc + 1) * QC)
                            nc.tensor.matmul(
                                pa[:, qs],
                                lhsT=vm[:, kt, h * P:(h + 1) * P],
                                rhs=et[kt][:, qs],
                                start=(kt == 0), stop=(kt == KO - 1))
                    # last head: per-qc epilogue so LN1's first chunk can
                    # start a bit earlier
                    for qs in ([slice(0, QC), slice(QC, NX)]
                               if h == H - 1 else [slice(0, NX)]):
                        nc.vector.tensor_mul(zt[:, h, qs], pa[:, qs], rc[:, qs])
                        nc.vector.scalar_tensor_tensor(
                            zt[:, h, qs], zt[:, h, qs], bv_sb[:, h:h + 1],
                            qt[:, h, qs], op0=ALU.add, op1=ALU.add)

                # software pipeline: head h's K proj / logits / exp overlap
                # head h-1's denominator (DVE tree + tiny ones-matmul) and
                # AV (PE).  Issue order is chosen per engine queue: the DVE
                # tree first (ready at iteration start), the PE denominator
                # matmuls after the logits matmuls (so the in-order PE queue
                # never waits on the tree).
                prev = None
                den_prev = None
                for h in range(H):
                    if prev is not None:
                        den_prev = denom_tree(h - 1, prev)
                    ktm_h = kproj(h)
                    et = logits_exp(h, ktm_h)
                    if DEBUG and h == 0:
                        nc.sync.dma_start(dbg["d_ktm0"], ktm_h)
                        nc.sync.dma_start(dbg["d_et00"], et[0])
                    if prev is not None:
                        rc_prev = denom_finish(h - 1, den_prev)
                        if DEBUG and h == 1:
                            nc.sync.dma_start(dbg["d_rc0"], rc_prev)
                        av_epi(h - 1, prev, rc_prev)
                    prev = et
                den_prev = denom_tree(H - 1, prev)
                rc_prev = denom_finish(H - 1, den_prev)
                av_epi(H - 1, prev, rc_prev)
                if DEBUG:
                    nc.sync.dma_start(dbg["d_zt"], zt)

            # ================= Phase 3: LN1 -> O proj + LN2 =================
            # LayerNorm over the model dim (partition direction): stats via
            # all-ones stationary matmuls (free broadcast), elementwise on
            # DVE, gamma/beta affine on ACT.
            with tc.tile_pool(name="sqp", bufs=2) as sqp, \
                 tc.tile_pool(name="stp", bufs=2) as stp, \
                 tc.tile_pool(name="out", bufs=2) as outp, \
                 tc.tile_pool(name="spp", bufs=2, space="PSUM") as spp, \
                 tc.tile_pool(name="gp3", bufs=4, space="PSUM") as pp3:

                def layernorm_chunk(in_sb, lo, hi, g_sb, b_sb, emit_out, lbl):
                    cs = hi - lo
                    qs = slice(lo, hi)
                    pmu = spp.tile([P, QC], F32, tag="pmu", name=f"pmu{lbl}{lo}")
                    for do in range(KO):
                        nc.tensor.matmul(pmu[:, :cs], lhsT=ones_bf,
                                         rhs=in_sb[:, do, qs],
                                         start=(do == 0), stop=(do == KO - 1))
                    sq = sqp.tile([P, KO, QC], BF16, tag="sq", name=f"sq{lbl}{lo}")
                    nc.scalar.square(sq[:, :, :cs], in_sb[:, :, qs])
                    ps2 = spp.tile([P, QC], F32, tag="ps2", name=f"ps2{lbl}{lo}")
                    for do in range(KO):
                        nc.tensor.matmul(ps2[:, :cs], lhsT=ones_bf,
                                         rhs=sq[:, do, :cs],
                                         start=(do == 0), stop=(do == KO - 1))
                    mu = stp.tile([P, QC], F32, tag="mu", name=f"mu{lbl}{lo}")[:, :cs]
                    nc.vector.tensor_scalar_mul(mu, pmu[:, :cs], 1.0 / DIM)
                    msq = stp.tile([P, QC], F32, tag="msq", name=f"msq{lbl}{lo}")[:, :cs]
                    nc.vector.tensor_mul(msq, mu, mu)
                    sd = stp.tile([P, QC], F32, tag="sd", name=f"sd{lbl}{lo}")[:, :cs]
                    nc.vector.scalar_tensor_tensor(
                        sd, ps2[:, :cs], 1.0 / DIM, msq,
                        op0=ALU.mult, op1=ALU.subtract)
                    nc.scalar.activation(sd, sd, AF.Sqrt, bias=eps_sb, scale=1.0)
                    rsig = stp.tile([P, QC], F32, tag="rsig", name=f"rsig{lbl}{lo}")[:, :cs]
                    nc.vector.reciprocal_approx_fast(rsig, sd)
                    mub = stp.tile([P, QC], BF16, tag="mub", name=f"mub{lbl}{lo}")[:, :cs]
                    nc.vector.tensor_copy(mub, mu)
                    rsb = stp.tile([P, QC], BF16, tag="rsb", name=f"rsb{lbl}{lo}")[:, :cs]
                    nc.vector.tensor_copy(rsb, rsig)
                    # t = (x - mu) * rsig over the whole [128, 8, cs] block
                    t = sqp.tile([P, KO, QC], BF16, tag="t", name=f"t{lbl}{lo}")
                    mu_b = mub.unsqueeze(1).broadcast_to([P, KO, cs])
                    rs_b = rsb.unsqueeze(1).broadcast_to([P, KO, cs])
                    nc.vector.tensor_sub(t[:, :, :cs], in_sb[:, :, qs], mu_b)
                    nc.vector.tensor_mul(t[:, :, :cs], t[:, :, :cs], rs_b)
                    for do in range(KO):
                        # out = t * g + b  (per-partition affine, DVE TS 4x)
                        emit_out(do, qs, t[:, do, :cs],
                                 g_sb[:, do:do + 1], b_sb[:, do:do + 1])

                o1t = actp.tile([P, KO, NX], BF16, tag="big", name="o1t")

                def emit_o1(do, qs, t, g_col, b_col):
                    nc.vector.tensor_scalar(o1t[:, do, qs], t, g_col, b_col,
                                            op0=ALU.mult, op1=ALU.add)

                z2t = actp.tile([P, KO, NX], BF16, tag="big", name="z2t")

                def oproj_qc(qc):
                    # HT[n, q] = sum_d Wo[d, n] O1T[d, q]; z2 = o1+relu(H+bo)
                    # (relu into a staging tile so the residual add is one
                    # merged DVE op per 512-token half)
                    qs = slice(qc * QC, (qc + 1) * QC)
                    ht = outp.tile([P, KO, QC], BF16, tag="ht", name=f"ht{qc}")
                    for no in range(KO):
                        po = pp3.tile([P, QC], F32, tag="po", name=f"po{no}{qc}")
                        for k in range(KO):
                            nc.tensor.matmul(
                                po,
                                lhsT=wo_sb[:, k, no * P:(no + 1) * P],
                                rhs=o1t[:, k, qs],
                                start=(k == 0), stop=(k == KO - 1))
                        nc.scalar.activation(ht[:, no, :], po, AF.Relu,
                                             bias=bo_sb[:, no:no + 1], scale=1.0)
                        if no == 3:   # first-half residual add starts mid-relu
                            nc.vector.tensor_add(z2t[:, 0:4, qs], ht[:, 0:4, :],
                                                 o1t[:, 0:4, qs])
                    nc.vector.tensor_add(z2t[:, 4:KO, qs], ht[:, 4:KO, :],
                                         o1t[:, 4:KO, qs])

                def make_emit_o2(lo, hi):
                    # affine writes into one staging tile; a single DMA per
                    # chunk replaces 8 per-do DMAs (issue + sem overhead)
                    cs = hi - lo
                    ost = outp.tile([P, KO, QC], BF16, tag="ost",
                                    name=f"ost{lo}")

                    def emit(do, qs, t, g_col, b_col):
                        nc.vector.tensor_scalar(ost[:, do, :cs], t, g_col,
                                                b_col, op0=ALU.mult, op1=ALU.add)
                        if do == KO - 1:
                            nc.sync.dma_start(ot3[:, :, lo:hi], ost[:, :, :cs])
                    return emit

                for qc in range(NQC):
                    layernorm_chunk(zt, qc * QC, (qc + 1) * QC,
                                    g1_sb, b1_sb, emit_o1, "a")
                if DEBUG:
                    nc.sync.dma_start(dbg["d_o1t"], o1t)
                # interleave the second O-proj half behind LN2's first chunk
                # so LN2's stats matmuls don't queue behind all of O-proj on
                # the in-order PE queue; LN2's shrinking chunks keep the
                # serial drain at the very end short
                oproj_qc(0)
                layernorm_chunk(z2t, 0, 512, g2_sb, b2_sb,
                                make_emit_o2(0, 512), "b")
                oproj_qc(1)
                if DEBUG:
                    nc.sync.dma_start(dbg["d_z2t"], z2t)
                for lo, hi in ((512, 768), (768, 1024)):
                    layernorm_chunk(z2t, lo, hi, g2_sb, b2_sb,
                                    make_emit_o2(lo, hi), "b")

    nc.compile()
    return nc


_CACHE = {}


def _get_nc():
    if "nc" not in _CACHE:
        _CACHE["nc"] = _build()
    return _CACHE["nc"]


def make_in_maps(X, Y, mask, Wq, bq, Wk, bk, Wv, bv, Wo, bo, g1, b1, g2, b2):
    bf = lambda a: np.ascontiguousarray(
        np.asarray(a, dtype=np.float32).astype(ml_dtypes.bfloat16))
    f = lambda a: np.ascontiguousarray(np.asarray(a, dtype=np.float32))
    shared = {
        "Wq": bf(Wq), "Wk": bf(Wk), "Wv": bf(Wv), "Wo": bf(Wo),
        "bq": f(bq), "bk": f(bk), "bv": f(bv), "bo": f(bo),
        "g1": f(g1), "b1": f(b1), "g2": f(g2), "b2": f(b2),
    }
    X = np.asarray(X, dtype=np.float32)
    Y = np.asarray(Y, dtype=np.float32)
    mask = np.asarray(mask)
    in_maps = []
    for b in range(8):
        mb = np.where(mask[b], np.float32(-1e4), np.float32(0.0)).astype(np.float32)
        in_maps.append({
            "XT": bf(X[b].T),
            "YT": bf(Y[b].T),
            "MB": mb,
            **shared,
        })
    return in_maps


def kernel(X, Y, mask, Wq, bq, Wk, bk, Wv, bv, Wo, bo, g1, b1, g2, b2,
           _trace=False):
    nc = _get_nc()
    in_maps = make_in_maps(X, Y, mask, Wq, bq, Wk, bk, Wv, bv, Wo, bo,
                           g1, b1, g2, b2)
    res = run_bass_kernel_spmd(nc, in_maps, core_ids=list(range(8)),
                               trace=_trace)
    out = np.stack([np.asarray(res.results[b]["OT"]).astype(np.float32).T
                    for b in range(8)])
    out = np.ascontiguousarray(out)
    if _trace:
        return out, res
    return out
